# revision 7
# baseline (speedup 1.0000x reference)
"""ALoraLinear on 8 TRN2 NeuronCores.

y = x @ W^T + b + mask ⊙ ((x @ A^T) @ B_w^T) * 2.0
  B=4, S=4096, D_IN=D_OUT=4096, R=32; mask = per-sample tail of the sequence.

Strategy (v3):
 1. Host folds the LoRA update into the weights: W' = W + 2·B_w@A. A token's
    output is x@W^T (unmasked) or x@W'^T (masked) — two dense GEMMs with no
    runtime LoRA path. Tokens are re-sorted host-side so each core is pure-W
    or pure-W' except ≤256 "minority" tokens per core, parked in 2 flex
    m-tiles and fixed up by a rank-32 correction with mask ∈ {0, ±2}.
 2. 10 of 32 k-subtiles run in fp8 e4m3 with perf_mode=DoubleRow: measured
    216 ns/MM at N=512 while contracting K=256 — a clean 2× over bf16.
    Emulated end-to-end rel err 0.0179 (gate 2e-2; HW matched emulation to
    <1e-4 at KF8=8).
 3. PSUM accumulates 32·y (weights pre-scaled ×32 so e4m3 sees RMS ~0.64
    instead of subnormal 0.02); bias (×32) is added by the vector engine at
    PSUM eviction; host divides the f32 output by 32 (exact).
 4. Quad super-groups: 4 m-tiles share one DR burst (DR-first, then 4×22
    bf16 k-MMs interleaved by k) — bf16↔fp8 mode transitions cost ~400 ns,
    amortized 4×. DR-first also gives the PE early work in the DMA-bound
    ramp; the x stream is issued in token-quarter passes matching the
    super-group consumption order.
"""

import numpy as np
import ml_dtypes

N_CORES = 8
B, S, D_IN, D_OUT, R = 4, 4096, 4096, 4096, 32
SCALING = 2.0
WSCALE = 32.0
P = 128
TOKC = (B * S) // N_CORES  # 2048 tokens per core
KT = D_IN // P  # 32 k-subtiles total
KF8 = 10  # k-subtiles in fp8 DoubleRow (must be even)
KBF = KT - KF8  # bf16 k-subtiles
NDR = KF8 // 2  # DoubleRow MMs per tile
NB = D_OUT // 512  # 8 n-blocks of 512
MT = TOKC // P  # 16 m-tiles of 128 tokens
SG = 4  # m-tiles per super-group (shared DR burst)
FLEXM = 2  # flex m-tiles (slots 0..255) carrying the LoRA fixup
FLEX = FLEXM * P
NW_CH = 3 if KBF % 3 == 0 else 2  # wt chunks per n-block
WCH = KBF // NW_CH  # bf16 k-subtiles per chunk DMA

_COMPILED = None


def _build():
    import concourse.bacc as bacc
    import concourse.mybir as mybir
    import concourse.tile as tile

    bf16 = mybir.dt.bfloat16
    f8 = mybir.dt.float8e4
    f32 = mybir.dt.float32
    DR = mybir.MatmulPerfMode.DoubleRow

    nc = bacc.Bacc("TRN2", target_bir_lowering=False, debug=False)

    xt_d = nc.dram_tensor("xt", [P, KBF, TOKC], bf16, kind="ExternalInput")
    xt8_d = nc.dram_tensor("xt8", [P, KF8, TOKC], f8, kind="ExternalInput")
    xtf_d = nc.dram_tensor("xtf", [P, KT, FLEX], bf16, kind="ExternalInput")
    wt_d = nc.dram_tensor("wt", [P, KBF, D_OUT], bf16, kind="ExternalInput")
    wt8_d = nc.dram_tensor("wt8", [P, KF8, D_OUT], f8, kind="ExternalInput")
    at_d = nc.dram_tensor("at", [P, KT, R], bf16, kind="ExternalInput")
    bwt_d = nc.dram_tensor("bwt", [R, D_OUT], bf16, kind="ExternalInput")
    maskv_d = nc.dram_tensor("maskv", [P, FLEX], bf16, kind="ExternalInput")
    bias_d = nc.dram_tensor("bias", [P, D_OUT], bf16, kind="ExternalInput")
    out_d = nc.dram_tensor("out", [TOKC, D_OUT], f32, kind="ExternalOutput")

    with tile.TileContext(nc) as tc:
        with (
            tc.tile_pool(name="const", bufs=1) as const,
            tc.tile_pool(name="xtp", bufs=1) as xtp,
            tc.tile_pool(name="wtp", bufs=NW_CH + 2) as wtp,
            tc.tile_pool(name="wt8p", bufs=2) as wt8p,
            tc.tile_pool(name="outp", bufs=4) as outp,
            tc.tile_pool(name="psum", bufs=7, space="PSUM") as psum,
            tc.tile_pool(name="psuma", bufs=1, space="PSUM") as psuma,
        ):
            at_sb = const.tile([P, KT, R], bf16, name="at_sb")
            xtf_sb = const.tile([P, KT, FLEX], bf16, name="xtf_sb")
            bwt_sb = const.tile([P, D_OUT], bf16, name="bwt_sb")
            maskv_sb = const.tile([P, FLEX], bf16, name="maskv_sb")
            bias_sb = const.tile([P, D_OUT], bf16, name="bias_sb")
            ut_sb = const.tile([P, FLEX], bf16, name="ut_sb")
            xt_sb = xtp.tile([P, KBF, TOKC], bf16, name="xt_sb")
            xt8_sb = xtp.tile([P, KF8, TOKC], f8, name="xt8_sb")

            def load_wt_chunk(n, c):
                wt = wtp.tile([P, WCH, 512], bf16, name="wt_sb")
                nc.sync.dma_start(
                    wt[:],
                    wt_d.ap()[:, c * WCH : (c + 1) * WCH, n * 512 : (n + 1) * 512],
                )
                return wt

            def load_wt8(n):
                w8 = wt8p.tile([P, KF8, 512], f8, name="wt8_sb")
                nc.sync.dma_start(w8[:], wt8_d.ap()[:, :, n * 512 : (n + 1) * 512])
                return w8

            # PE clock warmup: the HAM gate holds the PE at half clock until
            # ~3.4us of sustained activity; the first ~6us are DMA-only.
            warm_sb = const.tile([P, FLEX], bf16, name="warm_sb")
            nc.gpsimd.memset(warm_sb[:], 0.0)
            wps = psuma.tile([R, FLEX], f32, name="aps")
            for i in range(32):
                nc.tensor.matmul(
                    wps[:],
                    warm_sb[:, 0:R],
                    warm_sb[:],
                    start=(i == 0),
                    stop=(i == 31),
                )

            # zero partition strips 32..127 of ut/bwt so the tail matmul sees
            # no SBUF garbage (NaN·0 = NaN); compute engines can't address
            # partition ranges starting mid-strip, so 3 strips of 32
            for p0 in (32, 64, 96):
                nc.vector.memset(ut_sb[p0 : p0 + 32, :], 0.0)
                nc.vector.memset(bwt_sb[p0 : p0 + 32, :], 0.0)

            # sync preamble in PE-need order: act operands, fp8+bf16 weights
            # for n0, then eviction-time operands (bias/bwt needed ~30us in)
            nc.sync.dma_start(at_sb[:], at_d.ap()[:])
            nc.sync.dma_start(xtf_sb[:], xtf_d.ap()[:])
            nc.sync.dma_start(maskv_sb[:], maskv_d.ap()[:])
            wt8_0 = load_wt8(0)
            wt_chunks0 = [load_wt_chunk(0, c) for c in range(NW_CH)]
            nc.sync.dma_start(bwt_sb[0:R, :], bwt_d.ap()[:])
            nc.sync.dma_start(bias_sb[:], bias_d.ap()[:])

            # gpsimd x stream in consumption order: token-half passes — 1024
            # tokens = 2KB per-partition lines (full DMA bandwidth; 512-token
            # quarters would halve it). fp8 leads each half since the DR
            # burst leads each super-group.
            h0 = slice(0, TOKC // 2)
            h1 = slice(TOKC // 2, TOKC)
            nc.gpsimd.dma_start(xt8_sb[:, :, h0], xt8_d.ap()[:, :, h0])
            for k in range(KBF):
                nc.gpsimd.dma_start(
                    xt_sb[:, k : k + 1, h0], xt_d.ap()[:, k : k + 1, h0]
                )
            nc.gpsimd.dma_start(xt8_sb[:, :, h1], xt8_d.ap()[:, :, h1])
            for k in range(KBF):
                nc.gpsimd.dma_start(
                    xt_sb[:, k : k + 1, h1], xt_d.ap()[:, k : k + 1, h1]
                )

            # LoRA activation for flex tokens only: u^T = A_pad @ x_flex^T,
            # one PSUM bank, then mask·u on the vector engine
            aps = psuma.tile([R, FLEX], f32, name="aps")
            for k in range(KT):
                nc.tensor.matmul(
                    aps[:],
                    at_sb[:, k, :],
                    xtf_sb[:, k, :],
                    start=(k == 0),
                    stop=(k == KT - 1),
                )
            nc.vector.tensor_mul(ut_sb[0:R, :], aps[:], maskv_sb[0:R, :])

            def super_group(q, n, chunks, w8):
                """4 m-tiles (q*SG .. q*SG+3): DR burst first, then bf16
                k-loops interleaved by k, then tails/evictions."""
                nsl = slice(n * 512, (n + 1) * 512)
                ms = [q * SG + i for i in range(SG)]
                ps = [psum.tile([P, 512], f32, name="ps") for _ in range(SG)]
                for i, m in enumerate(ms):
                    msl = slice(m * P, (m + 1) * P)
                    for j in range(NDR):
                        nc.tensor.matmul(
                            ps[i][:],
                            xt8_sb[:, 2 * j : 2 * j + 2, msl],
                            w8[:, 2 * j : 2 * j + 2, :],
                            start=(j == 0),
                            stop=False,
                            perf_mode=DR,
                        )
                for k in range(KBF):
                    for i, m in enumerate(ms):
                        nc.tensor.matmul(
                            ps[i][:],
                            xt_sb[:, k, m * P : (m + 1) * P],
                            chunks[k // WCH][:, k % WCH, :],
                            start=False,
                            stop=(k == KBF - 1 and m >= FLEXM),
                        )
                for i, m in enumerate(ms):
                    msl = slice(m * P, (m + 1) * P)
                    if m < FLEXM:
                        # rank-32 LoRA fixup for minority tokens (mask ∈ {0,±2})
                        nc.tensor.matmul(
                            ps[i][:], ut_sb[:, msl], bwt_sb[:, nsl],
                            start=False, stop=True,
                        )
                    ot = outp.tile([P, 512], f32, name="ot")
                    # eviction fuses the (×32-scaled) bias add
                    nc.vector.tensor_add(ot[:], ps[i][:], bias_sb[:, nsl])
                    # scalar engine issues output DMAs so their sem-waits
                    # never stall the sync engine's wt-prefetch stream
                    nc.scalar.dma_start(out_d.ap()[msl, nsl], ot[:])

            for q in range(MT // SG):
                super_group(q, 0, wt_chunks0, wt8_0)
            for n in range(1, NB):
                wt8_n = load_wt8(n)
                wt_chunks = [load_wt_chunk(n, c) for c in range(NW_CH)]
                for q in range(MT // SG):
                    super_group(q, n, wt_chunks, wt8_n)

    nc.compile()
    return nc


def _get_compiled():
    global _COMPILED
    if _COMPILED is None:
        _COMPILED = _build()
    return _COMPILED


def _tile_kx(a_t: np.ndarray, dt) -> np.ndarray:
    """[K, F] -> partition-tiled [128, K/128, F], C-contiguous."""
    k, f = a_t.shape
    return np.ascontiguousarray(a_t.reshape(k // P, P, f).transpose(1, 0, 2)).astype(dt)


def _plan_permutation(offs):
    """Sort tokens so each core is pure-W or pure-W' except <=256 minority
    tokens parked in its first FLEX slots with mask ∈ {+2, -2}."""
    kk = np.minimum(offs, S)
    bnd = S - kk  # per-sample boundary; s >= bnd[i] is masked
    masked = np.zeros(B * S, dtype=bool)
    for i in range(B):
        masked[i * S + int(bnd[i]) : (i + 1) * S] = True
    unm = np.nonzero(~masked)[0]
    msk = np.nonzero(masked)[0]
    U = len(unm)

    n_w = None
    for cand in sorted(set([U // TOKC, -(-U // TOKC), round(U / TOKC)])):
        if cand < 0 or cand > N_CORES:
            continue
        delta = U - TOKC * cand
        if 0 <= delta <= FLEX * (N_CORES - cand) or (
            delta < 0 and -delta <= FLEX * cand
        ):
            n_w = cand
            break
    assert n_w is not None, f"no feasible core split for U={U}"
    delta = U - TOKC * n_w

    slot_token = np.empty((N_CORES, TOKC), dtype=np.int64)
    mask_val = np.zeros((N_CORES, FLEX), dtype=np.float32)
    core_w = np.zeros(N_CORES, dtype=bool)
    core_w[:n_w] = True

    iu = im = 0
    if delta >= 0:
        for c in range(n_w):  # W-cores: all unmasked
            slot_token[c] = unm[iu : iu + TOKC]
            iu += TOKC
        n_wp = N_CORES - n_w
        for j, c in enumerate(range(n_w, N_CORES)):
            share = delta // n_wp + (1 if j < delta % n_wp else 0)
            sl = unm[iu : iu + share]
            iu += share
            rest = msk[im : im + TOKC - share]
            im += TOKC - share
            slot_token[c] = np.concatenate([sl, rest])
            mask_val[c, :share] = -SCALING
    else:
        d = -delta
        for c in range(n_w, N_CORES):  # W'-cores: all masked
            slot_token[c] = msk[im : im + TOKC]
            im += TOKC
        for j, c in enumerate(range(n_w)):
            share = d // n_w + (1 if j < d % n_w else 0)
            sl = msk[im : im + share]
            im += share
            rest = unm[iu : iu + TOKC - share]
            iu += TOKC - share
            slot_token[c] = np.concatenate([sl, rest])
            mask_val[c, :share] = SCALING
    assert iu == len(unm) and im == len(msk)
    return slot_token, mask_val, core_w


def _prepare_in_maps(x, alora_offsets, W, b, A, B_w):
    bf = ml_dtypes.bfloat16
    f8 = ml_dtypes.float8_e4m3
    xf = np.asarray(x, dtype=np.float32).reshape(B * S, D_IN)
    W = np.asarray(W, dtype=np.float32)
    b = np.asarray(b, dtype=np.float32)
    A = np.asarray(A, dtype=np.float32)
    B_w = np.asarray(B_w, dtype=np.float32)
    offs = np.asarray(alora_offsets, dtype=np.int64)

    Wp = W + SCALING * (B_w @ A)
    slot_token, mask_val, core_w = _plan_permutation(offs)

    KSPLIT = KBF * P  # k-range split between bf16 and fp8

    def prep_w(Wm):
        Wt32 = Wm.T * WSCALE  # [D_IN, D_OUT]
        return _tile_kx(Wt32[:KSPLIT], bf), _tile_kx(Wt32[KSPLIT:], f8)

    wt_W, wt8_W = prep_w(W)
    wt_Wp, wt8_Wp = prep_w(Wp)
    at_np = _tile_kx(A.T, bf)  # [P, KT, R]
    bwt_np = (B_w.T * WSCALE).astype(bf)  # [R, D_OUT]
    bias_np = np.ascontiguousarray(
        np.broadcast_to((b * WSCALE).astype(bf), (P, D_OUT))
    )

    in_maps = []
    for c in range(N_CORES):
        xc = xf[slot_token[c]]  # [TOKC, D_IN]
        xt_np = _tile_kx(np.ascontiguousarray(xc[:, :KSPLIT].T), bf)
        xt8_np = _tile_kx(np.ascontiguousarray(xc[:, KSPLIT:].T), f8)
        xtf_np = _tile_kx(np.ascontiguousarray(xc[:FLEX].T), bf)
        maskv_np = np.ascontiguousarray(
            np.broadcast_to(mask_val[c].astype(bf), (P, FLEX))
        )
        wt_np, wt8_np = (wt_W, wt8_W) if core_w[c] else (wt_Wp, wt8_Wp)
        in_maps.append(
            {
                "xt": xt_np,
                "xt8": xt8_np,
                "xtf": xtf_np,
                "wt": wt_np,
                "wt8": wt8_np,
                "at": at_np,
                "bwt": bwt_np,
                "maskv": maskv_np,
                "bias": bias_np,
            }
        )
    return in_maps, slot_token


def _run(inputs: dict, trace: bool = False):
    from concourse.bass_utils import run_bass_kernel_spmd

    nc = _get_compiled()
    in_maps, slot_token = _prepare_in_maps(**inputs)
    res = None
    for attempt in range(3):
        try:
            res = run_bass_kernel_spmd(
                nc, in_maps, core_ids=list(range(N_CORES)), trace=trace
            )
            break
        except Exception:
            # transient device faults (e.g. NRT_EXEC_UNIT_UNRECOVERABLE)
            # clear on retry; re-raise only if persistent
            if attempt == 2:
                raise
    out = np.empty((B * S, D_OUT), dtype=np.float32)
    for c in range(N_CORES):
        out[slot_token[c]] = res.results[c]["out"]
    out /= WSCALE  # exact power-of-2 rescale of the scale-32 PSUM
    return out.reshape(B, S, D_OUT), res


def kernel(x, alora_offsets, W, b, A, B_w) -> np.ndarray:
    out, _ = _run(
        {"x": x, "alora_offsets": alora_offsets, "W": W, "b": b, "A": A, "B_w": B_w}
    )
    return out


# revision 15
# speedup vs baseline: 1.0229x; 1.0229x over previous
"""ALoraLinear on 8 TRN2 NeuronCores.

y = x @ W^T + b + mask ⊙ ((x @ A^T) @ B_w^T) * 2.0
  B=4, S=4096, D_IN=D_OUT=4096, R=32; mask = per-sample tail of the sequence.

Strategy (v3):
 1. Host folds the LoRA update into the weights: W' = W + 2·B_w@A. A token's
    output is x@W^T (unmasked) or x@W'^T (masked) — two dense GEMMs with no
    runtime LoRA path. Tokens are re-sorted host-side so each core is pure-W
    or pure-W' except ≤256 "minority" tokens per core, parked in 2 flex
    m-tiles and fixed up by a rank-32 correction with mask ∈ {0, ±2}.
 2. 10 of 32 k-subtiles run in fp8 e4m3 with perf_mode=DoubleRow: measured
    216 ns/MM at N=512 while contracting K=256 — a clean 2× over bf16.
    Emulated end-to-end rel err 0.0179 (gate 2e-2; HW matched emulation to
    <1e-4 at KF8=8).
 3. PSUM accumulates 32·y (weights pre-scaled ×32 so e4m3 sees RMS ~0.64
    instead of subnormal 0.02); bias (×32) is added by the vector engine at
    PSUM eviction; host divides the f32 output by 32 (exact).
 4. Quad super-groups: 4 m-tiles share one DR burst (DR-first, then 4×22
    bf16 k-MMs interleaved by k) — bf16↔fp8 mode transitions cost ~400 ns,
    amortized 4×. DR-first also gives the PE early work in the DMA-bound
    ramp; the x stream is issued in token-quarter passes matching the
    super-group consumption order.
"""

import numpy as np
import ml_dtypes

N_CORES = 8
B, S, D_IN, D_OUT, R = 4, 4096, 4096, 4096, 32
SCALING = 2.0
WSCALE = 32.0
P = 128
TOKC = (B * S) // N_CORES  # 2048 tokens per core
KT = D_IN // P  # 32 k-subtiles total
KF8 = 10  # k-subtiles in fp8 DoubleRow (must be even)
KBF = KT - KF8  # bf16 k-subtiles
NDR = KF8 // 2  # DoubleRow MMs per tile
NB = D_OUT // 512  # 8 n-blocks of 512
MT = TOKC // P  # 16 m-tiles of 128 tokens
SG = 4  # m-tiles per super-group (shared DR burst)
FLEXM = 2  # flex m-tiles (slots 0..255) carrying the LoRA fixup
FLEX = FLEXM * P
NW_CH = 3 if KBF % 3 == 0 else 2  # wt chunks per n-block
WCH = KBF // NW_CH  # bf16 k-subtiles per chunk DMA

_COMPILED = None


def _build():
    import concourse.bacc as bacc
    import concourse.mybir as mybir
    import concourse.tile as tile

    bf16 = mybir.dt.bfloat16
    f8 = mybir.dt.float8e4
    f32 = mybir.dt.float32
    DR = mybir.MatmulPerfMode.DoubleRow

    nc = bacc.Bacc("TRN2", target_bir_lowering=False, debug=False)

    # block-major layouts: every DMA below is per-partition contiguous
    xt_d = nc.dram_tensor("xt", [P, SG, KBF, 512], bf16, kind="ExternalInput")
    xt8_d = nc.dram_tensor("xt8", [P, SG, KF8, 512], f8, kind="ExternalInput")
    xtf_d = nc.dram_tensor("xtf", [P, KT, FLEX], bf16, kind="ExternalInput")
    wt_d = nc.dram_tensor("wt", [P, NB, KBF, 512], bf16, kind="ExternalInput")
    wt8_d = nc.dram_tensor("wt8", [P, NB, KF8, 512], f8, kind="ExternalInput")
    at_d = nc.dram_tensor("at", [P, KT, R], bf16, kind="ExternalInput")
    bwt_d = nc.dram_tensor("bwt", [R, D_OUT], bf16, kind="ExternalInput")
    maskv_d = nc.dram_tensor("maskv", [P, FLEX], bf16, kind="ExternalInput")
    bias_d = nc.dram_tensor("bias", [P, D_OUT], bf16, kind="ExternalInput")
    out_d = nc.dram_tensor("out", [TOKC, D_OUT], f32, kind="ExternalOutput")

    with tile.TileContext(nc) as tc:
        with (
            tc.tile_pool(name="const", bufs=1) as const,
            tc.tile_pool(name="xtp", bufs=1) as xtp,
            tc.tile_pool(name="wtp", bufs=NW_CH + 2) as wtp,
            tc.tile_pool(name="wt8p", bufs=2) as wt8p,
            tc.tile_pool(name="outp", bufs=4) as outp,
            tc.tile_pool(name="psum", bufs=7, space="PSUM") as psum,
            tc.tile_pool(name="psuma", bufs=1, space="PSUM") as psuma,
        ):
            at_sb = const.tile([P, KT, R], bf16, name="at_sb")
            xtf_sb = const.tile([P, KT, FLEX], bf16, name="xtf_sb")
            bwt_sb = const.tile([P, D_OUT], bf16, name="bwt_sb")
            maskv_sb = const.tile([P, FLEX], bf16, name="maskv_sb")
            bias_sb = const.tile([P, D_OUT], bf16, name="bias_sb")
            ut_sb = const.tile([P, FLEX], bf16, name="ut_sb")
            xt_sb = xtp.tile([P, SG, KBF, 512], bf16, name="xt_sb")
            xt8_sb = xtp.tile([P, SG, KF8, 512], f8, name="xt8_sb")

            def load_wt_chunk(n, c):
                wt = wtp.tile([P, WCH, 512], bf16, name="wt_sb")
                nc.sync.dma_start(
                    wt[:], wt_d.ap()[:, n : n + 1, c * WCH : (c + 1) * WCH, :]
                )
                return wt

            def load_wt8(n):
                w8 = wt8p.tile([P, KF8, 512], f8, name="wt8_sb")
                nc.sync.dma_start(w8[:], wt8_d.ap()[:, n : n + 1, :, :])
                return w8

            # PE clock warmup: the HAM gate holds the PE at half clock until
            # ~3.4us of sustained activity; the first ~6us are DMA-only.
            warm_sb = const.tile([P, FLEX], bf16, name="warm_sb")
            nc.gpsimd.memset(warm_sb[:], 0.0)
            wps = psuma.tile([R, FLEX], f32, name="aps")
            for i in range(32):
                nc.tensor.matmul(
                    wps[:],
                    warm_sb[:, 0:R],
                    warm_sb[:],
                    start=(i == 0),
                    stop=(i == 31),
                )

            # zero partition strips 32..127 of ut/bwt so the tail matmul sees
            # no SBUF garbage (NaN·0 = NaN); compute engines can't address
            # partition ranges starting mid-strip, so 3 strips of 32
            for p0 in (32, 64, 96):
                nc.vector.memset(ut_sb[p0 : p0 + 32, :], 0.0)
                nc.vector.memset(bwt_sb[p0 : p0 + 32, :], 0.0)

            # sync preamble in PE-need order: act operands, fp8+bf16 weights
            # for n0, then eviction-time operands (bias/bwt needed ~30us in)
            nc.sync.dma_start(at_sb[:], at_d.ap()[:])
            nc.sync.dma_start(xtf_sb[:], xtf_d.ap()[:])
            nc.sync.dma_start(maskv_sb[:], maskv_d.ap()[:])
            wt8_0 = load_wt8(0)
            wt_chunks0 = [load_wt_chunk(0, c) for c in range(NW_CH)]
            nc.sync.dma_start(bwt_sb[0:R, :], bwt_d.ap()[:])
            nc.sync.dma_start(bias_sb[:], bias_d.ap()[:])

            # gpsimd x stream in consumption order (super-group q consumes
            # token quarter q). Quarter 0 arrives k-granular so the ramp
            # chases it MM-by-MM; quarters 1..3 are one contiguous DMA each
            # (fewer sequencer issues — the issue rate, ~0.8us each, would
            # otherwise gate the stream).
            nc.gpsimd.dma_start(xt8_sb[:, 0:1], xt8_d.ap()[:, 0:1])
            for k in range(KBF):
                nc.gpsimd.dma_start(
                    xt_sb[:, 0:1, k : k + 1, :], xt_d.ap()[:, 0:1, k : k + 1, :]
                )
            nc.gpsimd.dma_start(xt8_sb[:, 1:SG], xt8_d.ap()[:, 1:SG])
            for q in range(1, SG):
                nc.gpsimd.dma_start(xt_sb[:, q : q + 1], xt_d.ap()[:, q : q + 1])

            # LoRA activation for flex tokens only: u^T = A_pad @ x_flex^T,
            # one PSUM bank, then mask·u on the vector engine
            aps = psuma.tile([R, FLEX], f32, name="aps")
            for k in range(KT):
                nc.tensor.matmul(
                    aps[:],
                    at_sb[:, k, :],
                    xtf_sb[:, k, :],
                    start=(k == 0),
                    stop=(k == KT - 1),
                )
            nc.vector.tensor_mul(ut_sb[0:R, :], aps[:], maskv_sb[0:R, :])

            def super_group(q, n, chunks, w8):
                """4 m-tiles (q*SG .. q*SG+3): DR burst first, then bf16
                k-loops interleaved by k, then tails/evictions."""
                nsl = slice(n * 512, (n + 1) * 512)
                ms = [q * SG + i for i in range(SG)]
                ps = [psum.tile([P, 512], f32, name="ps") for _ in range(SG)]
                for i, m in enumerate(ms):
                    rsl = slice(i * P, (i + 1) * P)
                    for j in range(NDR):
                        nc.tensor.matmul(
                            ps[i][:],
                            xt8_sb[:, q, 2 * j : 2 * j + 2, rsl],
                            w8[:, 2 * j : 2 * j + 2, :],
                            start=(j == 0),
                            stop=False,
                            perf_mode=DR,
                        )
                for k in range(KBF):
                    for i, m in enumerate(ms):
                        nc.tensor.matmul(
                            ps[i][:],
                            xt_sb[:, q, k, i * P : (i + 1) * P],
                            chunks[k // WCH][:, k % WCH, :],
                            start=False,
                            stop=(k == KBF - 1 and m >= FLEXM),
                        )
                for i, m in enumerate(ms):
                    msl = slice(m * P, (m + 1) * P)
                    if m < FLEXM:
                        # rank-32 LoRA fixup for minority tokens (mask ∈ {0,±2})
                        nc.tensor.matmul(
                            ps[i][:], ut_sb[:, msl], bwt_sb[:, nsl],
                            start=False, stop=True,
                        )
                    ot = outp.tile([P, 512], f32, name="ot")
                    # eviction fuses the (×32-scaled) bias add
                    nc.vector.tensor_add(ot[:], ps[i][:], bias_sb[:, nsl])
                    # scalar engine issues output DMAs so their sem-waits
                    # never stall the sync engine's wt-prefetch stream
                    nc.scalar.dma_start(out_d.ap()[msl, nsl], ot[:])

            for q in range(MT // SG):
                super_group(q, 0, wt_chunks0, wt8_0)
            for n in range(1, NB):
                wt8_n = load_wt8(n)
                wt_chunks = [load_wt_chunk(n, c) for c in range(NW_CH)]
                for q in range(MT // SG):
                    super_group(q, n, wt_chunks, wt8_n)

    nc.compile()
    return nc


def _get_compiled():
    global _COMPILED
    if _COMPILED is None:
        _COMPILED = _build()
    return _COMPILED


def _tile_kx(a_t: np.ndarray, dt) -> np.ndarray:
    """[K, F] -> partition-tiled [128, K/128, F], C-contiguous."""
    k, f = a_t.shape
    return np.ascontiguousarray(a_t.reshape(k // P, P, f).transpose(1, 0, 2)).astype(dt)


def _plan_permutation(offs):
    """Sort tokens so each core is pure-W or pure-W' except <=256 minority
    tokens parked in its first FLEX slots with mask ∈ {+2, -2}."""
    kk = np.minimum(offs, S)
    bnd = S - kk  # per-sample boundary; s >= bnd[i] is masked
    masked = np.zeros(B * S, dtype=bool)
    for i in range(B):
        masked[i * S + int(bnd[i]) : (i + 1) * S] = True
    unm = np.nonzero(~masked)[0]
    msk = np.nonzero(masked)[0]
    U = len(unm)

    n_w = None
    for cand in sorted(set([U // TOKC, -(-U // TOKC), round(U / TOKC)])):
        if cand < 0 or cand > N_CORES:
            continue
        delta = U - TOKC * cand
        if 0 <= delta <= FLEX * (N_CORES - cand) or (
            delta < 0 and -delta <= FLEX * cand
        ):
            n_w = cand
            break
    assert n_w is not None, f"no feasible core split for U={U}"
    delta = U - TOKC * n_w

    slot_token = np.empty((N_CORES, TOKC), dtype=np.int64)
    mask_val = np.zeros((N_CORES, FLEX), dtype=np.float32)
    core_w = np.zeros(N_CORES, dtype=bool)
    core_w[:n_w] = True

    iu = im = 0
    if delta >= 0:
        for c in range(n_w):  # W-cores: all unmasked
            slot_token[c] = unm[iu : iu + TOKC]
            iu += TOKC
        n_wp = N_CORES - n_w
        for j, c in enumerate(range(n_w, N_CORES)):
            share = delta // n_wp + (1 if j < delta % n_wp else 0)
            sl = unm[iu : iu + share]
            iu += share
            rest = msk[im : im + TOKC - share]
            im += TOKC - share
            slot_token[c] = np.concatenate([sl, rest])
            mask_val[c, :share] = -SCALING
    else:
        d = -delta
        for c in range(n_w, N_CORES):  # W'-cores: all masked
            slot_token[c] = msk[im : im + TOKC]
            im += TOKC
        for j, c in enumerate(range(n_w)):
            share = d // n_w + (1 if j < d % n_w else 0)
            sl = msk[im : im + share]
            im += share
            rest = unm[iu : iu + TOKC - share]
            iu += TOKC - share
            slot_token[c] = np.concatenate([sl, rest])
            mask_val[c, :share] = SCALING
    assert iu == len(unm) and im == len(msk)
    return slot_token, mask_val, core_w


def _prepare_in_maps(x, alora_offsets, W, b, A, B_w):
    bf = ml_dtypes.bfloat16
    f8 = ml_dtypes.float8_e4m3
    xf = np.asarray(x, dtype=np.float32).reshape(B * S, D_IN)
    W = np.asarray(W, dtype=np.float32)
    b = np.asarray(b, dtype=np.float32)
    A = np.asarray(A, dtype=np.float32)
    B_w = np.asarray(B_w, dtype=np.float32)
    offs = np.asarray(alora_offsets, dtype=np.int64)

    Wp = W + SCALING * (B_w @ A)
    slot_token, mask_val, core_w = _plan_permutation(offs)

    KSPLIT = KBF * P  # k-range split between bf16 and fp8

    def blockmajor(a, nblk):
        # [P, K, nblk*512] -> [P, nblk, K, 512] contiguous
        p, k, f = a.shape
        return np.ascontiguousarray(
            a.reshape(p, k, nblk, 512).transpose(0, 2, 1, 3)
        )

    def prep_w(Wm):
        Wt32 = Wm.T * WSCALE  # [D_IN, D_OUT]
        return (
            blockmajor(_tile_kx(Wt32[:KSPLIT], bf), NB),
            blockmajor(_tile_kx(Wt32[KSPLIT:], f8), NB),
        )

    wt_W, wt8_W = prep_w(W)
    wt_Wp, wt8_Wp = prep_w(Wp)
    at_np = _tile_kx(A.T, bf)  # [P, KT, R]
    bwt_np = (B_w.T * WSCALE).astype(bf)  # [R, D_OUT]
    bias_np = np.ascontiguousarray(
        np.broadcast_to((b * WSCALE).astype(bf), (P, D_OUT))
    )

    in_maps = []
    for c in range(N_CORES):
        xc = xf[slot_token[c]]  # [TOKC, D_IN]
        xt_np = blockmajor(_tile_kx(np.ascontiguousarray(xc[:, :KSPLIT].T), bf), SG)
        xt8_np = blockmajor(
            _tile_kx(np.ascontiguousarray(xc[:, KSPLIT:].T), f8), SG
        )
        xtf_np = _tile_kx(np.ascontiguousarray(xc[:FLEX].T), bf)
        maskv_np = np.ascontiguousarray(
            np.broadcast_to(mask_val[c].astype(bf), (P, FLEX))
        )
        wt_np, wt8_np = (wt_W, wt8_W) if core_w[c] else (wt_Wp, wt8_Wp)
        in_maps.append(
            {
                "xt": xt_np,
                "xt8": xt8_np,
                "xtf": xtf_np,
                "wt": wt_np,
                "wt8": wt8_np,
                "at": at_np,
                "bwt": bwt_np,
                "maskv": maskv_np,
                "bias": bias_np,
            }
        )
    return in_maps, slot_token


def _run(inputs: dict, trace: bool = False):
    from concourse.bass_utils import run_bass_kernel_spmd

    nc = _get_compiled()
    in_maps, slot_token = _prepare_in_maps(**inputs)
    res = None
    for attempt in range(3):
        try:
            res = run_bass_kernel_spmd(
                nc, in_maps, core_ids=list(range(N_CORES)), trace=trace
            )
            break
        except Exception:
            # transient device faults (e.g. NRT_EXEC_UNIT_UNRECOVERABLE)
            # clear on retry; re-raise only if persistent
            if attempt == 2:
                raise
    out = np.empty((B * S, D_OUT), dtype=np.float32)
    for c in range(N_CORES):
        out[slot_token[c]] = res.results[c]["out"]
    out /= WSCALE  # exact power-of-2 rescale of the scale-32 PSUM
    return out.reshape(B, S, D_OUT), res


def kernel(x, alora_offsets, W, b, A, B_w) -> np.ndarray:
    out, _ = _run(
        {"x": x, "alora_offsets": alora_offsets, "W": W, "b": b, "A": A, "B_w": B_w}
    )
    return out


# revision 16
# speedup vs baseline: 1.0245x; 1.0016x over previous
"""ALoraLinear on 8 TRN2 NeuronCores.

y = x @ W^T + b + mask ⊙ ((x @ A^T) @ B_w^T) * 2.0
  B=4, S=4096, D_IN=D_OUT=4096, R=32; mask = per-sample tail of the sequence.

Strategy (v3):
 1. Host folds the LoRA update into the weights: W' = W + 2·B_w@A. A token's
    output is x@W^T (unmasked) or x@W'^T (masked) — two dense GEMMs with no
    runtime LoRA path. Tokens are re-sorted host-side so each core is pure-W
    or pure-W' except ≤256 "minority" tokens per core, parked in 2 flex
    m-tiles and fixed up by a rank-32 correction with mask ∈ {0, ±2}.
 2. 10 of 32 k-subtiles run in fp8 e4m3 with perf_mode=DoubleRow: measured
    216 ns/MM at N=512 while contracting K=256 — a clean 2× over bf16.
    Emulated end-to-end rel err 0.0179 (gate 2e-2; HW matched emulation to
    <1e-4 at KF8=8).
 3. PSUM accumulates 32·y (weights pre-scaled ×32 so e4m3 sees RMS ~0.64
    instead of subnormal 0.02); bias (×32) is added by the vector engine at
    PSUM eviction; host divides the f32 output by 32 (exact).
 4. Quad super-groups: 4 m-tiles share one DR burst (DR-first, then 4×22
    bf16 k-MMs interleaved by k) — bf16↔fp8 mode transitions cost ~400 ns,
    amortized 4×. DR-first also gives the PE early work in the DMA-bound
    ramp; the x stream is issued in token-quarter passes matching the
    super-group consumption order.
"""

import numpy as np
import ml_dtypes

N_CORES = 8
B, S, D_IN, D_OUT, R = 4, 4096, 4096, 4096, 32
SCALING = 2.0
WSCALE = 32.0
P = 128
TOKC = (B * S) // N_CORES  # 2048 tokens per core
KT = D_IN // P  # 32 k-subtiles total
KF8 = 10  # k-subtiles in fp8 DoubleRow (must be even)
KBF = KT - KF8  # bf16 k-subtiles
NDR = KF8 // 2  # DoubleRow MMs per tile
NB = D_OUT // 512  # 8 n-blocks of 512
MT = TOKC // P  # 16 m-tiles of 128 tokens
SG = 4  # m-tiles per super-group (shared DR burst)
FLEXM = 2  # flex m-tiles (slots 0..255) carrying the LoRA fixup
FLEX = FLEXM * P
NW_CH = 3 if KBF % 3 == 0 else 2  # wt chunks per n-block
WCH = KBF // NW_CH  # bf16 k-subtiles per chunk DMA

_COMPILED = None


def _build():
    import concourse.bacc as bacc
    import concourse.mybir as mybir
    import concourse.tile as tile

    bf16 = mybir.dt.bfloat16
    f8 = mybir.dt.float8e4
    f32 = mybir.dt.float32
    DR = mybir.MatmulPerfMode.DoubleRow

    nc = bacc.Bacc("TRN2", target_bir_lowering=False, debug=False)

    # block-major layouts: every DMA below is per-partition contiguous
    xt_d = nc.dram_tensor("xt", [P, SG, KBF, 512], bf16, kind="ExternalInput")
    xt8_d = nc.dram_tensor("xt8", [P, SG, KF8, 512], f8, kind="ExternalInput")
    xtf_d = nc.dram_tensor("xtf", [P, KT, FLEX], bf16, kind="ExternalInput")
    wt_d = nc.dram_tensor("wt", [P, NB, KBF, 512], bf16, kind="ExternalInput")
    wt8_d = nc.dram_tensor("wt8", [P, NB, KF8, 512], f8, kind="ExternalInput")
    at_d = nc.dram_tensor("at", [P, KT, R], bf16, kind="ExternalInput")
    bwt_d = nc.dram_tensor("bwt", [R, D_OUT], bf16, kind="ExternalInput")
    maskv_d = nc.dram_tensor("maskv", [P, FLEX], bf16, kind="ExternalInput")
    bias_d = nc.dram_tensor("bias", [P, D_OUT], bf16, kind="ExternalInput")
    out_d = nc.dram_tensor("out", [TOKC, D_OUT], f32, kind="ExternalOutput")

    with tile.TileContext(nc) as tc:
        with (
            tc.tile_pool(name="const", bufs=1) as const,
            tc.tile_pool(name="xtp", bufs=1) as xtp,
            tc.tile_pool(name="wtp", bufs=NW_CH + 2) as wtp,
            tc.tile_pool(name="wt8p", bufs=2) as wt8p,
            tc.tile_pool(name="outp", bufs=4) as outp,
            tc.tile_pool(name="psum", bufs=7, space="PSUM") as psum,
            tc.tile_pool(name="psuma", bufs=1, space="PSUM") as psuma,
        ):
            at_sb = const.tile([P, KT, R], bf16, name="at_sb")
            xtf_sb = const.tile([P, KT, FLEX], bf16, name="xtf_sb")
            bwt_sb = const.tile([P, D_OUT], bf16, name="bwt_sb")
            maskv_sb = const.tile([P, FLEX], bf16, name="maskv_sb")
            bias_sb = const.tile([P, D_OUT], bf16, name="bias_sb")
            ut_sb = const.tile([P, FLEX], bf16, name="ut_sb")
            xt_sb = xtp.tile([P, SG, KBF, 512], bf16, name="xt_sb")
            xt8_sb = xtp.tile([P, SG, KF8, 512], f8, name="xt8_sb")

            def load_wt_chunk(n, c):
                wt = wtp.tile([P, WCH, 512], bf16, name="wt_sb")
                nc.sync.dma_start(
                    wt[:], wt_d.ap()[:, n : n + 1, c * WCH : (c + 1) * WCH, :]
                )
                return wt

            def load_wt8(n):
                w8 = wt8p.tile([P, KF8, 512], f8, name="wt8_sb")
                nc.sync.dma_start(w8[:], wt8_d.ap()[:, n : n + 1, :, :])
                return w8

            # PE clock warmup: the HAM gate holds the PE at half clock until
            # ~3.4us of sustained activity; the first ~6us are DMA-only.
            warm_sb = const.tile([P, FLEX], bf16, name="warm_sb")
            nc.gpsimd.memset(warm_sb[:], 0.0)
            wps = psuma.tile([R, FLEX], f32, name="aps")
            for i in range(32):
                nc.tensor.matmul(
                    wps[:],
                    warm_sb[:, 0:R],
                    warm_sb[:],
                    start=(i == 0),
                    stop=(i == 31),
                )

            # zero partition strips 32..127 of ut/bwt so the tail matmul sees
            # no SBUF garbage (NaN·0 = NaN); compute engines can't address
            # partition ranges starting mid-strip, so 3 strips of 32
            for p0 in (32, 64, 96):
                nc.vector.memset(ut_sb[p0 : p0 + 32, :], 0.0)
                nc.vector.memset(bwt_sb[p0 : p0 + 32, :], 0.0)

            # sync preamble in PE-need order: act operands, fp8+bf16 weights
            # for n0, then eviction-time operands (bias/bwt needed ~30us in)
            nc.sync.dma_start(at_sb[:], at_d.ap()[:])
            nc.sync.dma_start(xtf_sb[:], xtf_d.ap()[:])
            nc.sync.dma_start(maskv_sb[:], maskv_d.ap()[:])
            wt8_0 = load_wt8(0)
            wt_chunks0 = [load_wt_chunk(0, c) for c in range(NW_CH)]
            nc.sync.dma_start(bwt_sb[0:R, :], bwt_d.ap()[:])
            nc.sync.dma_start(bias_sb[:], bias_d.ap()[:])

            # gpsimd x stream in consumption order (super-group q consumes
            # token quarter q). Quarter 0 arrives k-granular so the ramp
            # chases it MM-by-MM; quarters 1..3 are one contiguous DMA each
            # (fewer sequencer issues — the issue rate, ~0.8us each, would
            # otherwise gate the stream).
            nc.gpsimd.dma_start(xt8_sb[:, 0:1], xt8_d.ap()[:, 0:1])
            for k in range(KBF):
                nc.gpsimd.dma_start(
                    xt_sb[:, 0:1, k : k + 1, :], xt_d.ap()[:, 0:1, k : k + 1, :]
                )
            nc.gpsimd.dma_start(xt8_sb[:, 1:SG], xt8_d.ap()[:, 1:SG])
            for q in range(1, SG):
                nc.gpsimd.dma_start(xt_sb[:, q : q + 1], xt_d.ap()[:, q : q + 1])

            # LoRA activation for flex tokens only: u^T = A_pad @ x_flex^T,
            # one PSUM bank, then mask·u on the vector engine
            aps = psuma.tile([R, FLEX], f32, name="aps")
            for k in range(KT):
                nc.tensor.matmul(
                    aps[:],
                    at_sb[:, k, :],
                    xtf_sb[:, k, :],
                    start=(k == 0),
                    stop=(k == KT - 1),
                )
            nc.vector.tensor_mul(ut_sb[0:R, :], aps[:], maskv_sb[0:R, :])

            def super_group(q, n, chunks, w8):
                """4 m-tiles (q*SG .. q*SG+3): DR burst first, then bf16
                k-loops interleaved by k, then tails/evictions."""
                nsl = slice(n * 512, (n + 1) * 512)
                ms = [q * SG + i for i in range(SG)]
                ps = [psum.tile([P, 512], f32, name="ps") for _ in range(SG)]
                for i, m in enumerate(ms):
                    rsl = slice(i * P, (i + 1) * P)
                    for j in range(NDR):
                        nc.tensor.matmul(
                            ps[i][:],
                            xt8_sb[:, q, 2 * j : 2 * j + 2, rsl],
                            w8[:, 2 * j : 2 * j + 2, :],
                            start=(j == 0),
                            stop=False,
                            perf_mode=DR,
                        )
                for k in range(KBF):
                    for i, m in enumerate(ms):
                        nc.tensor.matmul(
                            ps[i][:],
                            xt_sb[:, q, k, i * P : (i + 1) * P],
                            chunks[k // WCH][:, k % WCH, :],
                            start=False,
                            stop=(k == KBF - 1 and m >= FLEXM),
                        )
                for i, m in enumerate(ms):
                    msl = slice(m * P, (m + 1) * P)
                    if m < FLEXM:
                        # rank-32 LoRA fixup for minority tokens (mask ∈ {0,±2})
                        nc.tensor.matmul(
                            ps[i][:], ut_sb[:, msl], bwt_sb[:, nsl],
                            start=False, stop=True,
                        )
                    ot = outp.tile([P, 512], f32, name="ot")
                    # eviction fuses the (×32-scaled) bias add
                    nc.vector.tensor_add(ot[:], ps[i][:], bias_sb[:, nsl])
                    # scalar engine issues output DMAs so their sem-waits
                    # never stall the sync engine's wt-prefetch stream
                    nc.scalar.dma_start(out_d.ap()[msl, nsl], ot[:])

            # phase 1: token quarter 0 across all n-blocks — needs only
            # 3.5MB of x up front, weights stream as consumed, so the PE
            # starts ~40us before the full x stream has landed
            super_group(0, 0, wt_chunks0, wt8_0)
            for n in range(1, NB):
                wt8_n = load_wt8(n)
                wt_chunks = [load_wt_chunk(n, c) for c in range(NW_CH)]
                super_group(0, n, wt_chunks, wt8_n)
            # phase 2: quarters 1..3, n-outer (weights re-streamed once more;
            # ~27MB extra DMA, fully hidden under ~560us of PE work)
            for n in range(NB):
                wt8_n = load_wt8(n)
                wt_chunks = [load_wt_chunk(n, c) for c in range(NW_CH)]
                for q in range(1, MT // SG):
                    super_group(q, n, wt_chunks, wt8_n)

    nc.compile()
    return nc


def _get_compiled():
    global _COMPILED
    if _COMPILED is None:
        _COMPILED = _build()
    return _COMPILED


def _tile_kx(a_t: np.ndarray, dt) -> np.ndarray:
    """[K, F] -> partition-tiled [128, K/128, F], C-contiguous."""
    k, f = a_t.shape
    return np.ascontiguousarray(a_t.reshape(k // P, P, f).transpose(1, 0, 2)).astype(dt)


def _plan_permutation(offs):
    """Sort tokens so each core is pure-W or pure-W' except <=256 minority
    tokens parked in its first FLEX slots with mask ∈ {+2, -2}."""
    kk = np.minimum(offs, S)
    bnd = S - kk  # per-sample boundary; s >= bnd[i] is masked
    masked = np.zeros(B * S, dtype=bool)
    for i in range(B):
        masked[i * S + int(bnd[i]) : (i + 1) * S] = True
    unm = np.nonzero(~masked)[0]
    msk = np.nonzero(masked)[0]
    U = len(unm)

    n_w = None
    for cand in sorted(set([U // TOKC, -(-U // TOKC), round(U / TOKC)])):
        if cand < 0 or cand > N_CORES:
            continue
        delta = U - TOKC * cand
        if 0 <= delta <= FLEX * (N_CORES - cand) or (
            delta < 0 and -delta <= FLEX * cand
        ):
            n_w = cand
            break
    assert n_w is not None, f"no feasible core split for U={U}"
    delta = U - TOKC * n_w

    slot_token = np.empty((N_CORES, TOKC), dtype=np.int64)
    mask_val = np.zeros((N_CORES, FLEX), dtype=np.float32)
    core_w = np.zeros(N_CORES, dtype=bool)
    core_w[:n_w] = True

    iu = im = 0
    if delta >= 0:
        for c in range(n_w):  # W-cores: all unmasked
            slot_token[c] = unm[iu : iu + TOKC]
            iu += TOKC
        n_wp = N_CORES - n_w
        for j, c in enumerate(range(n_w, N_CORES)):
            share = delta // n_wp + (1 if j < delta % n_wp else 0)
            sl = unm[iu : iu + share]
            iu += share
            rest = msk[im : im + TOKC - share]
            im += TOKC - share
            slot_token[c] = np.concatenate([sl, rest])
            mask_val[c, :share] = -SCALING
    else:
        d = -delta
        for c in range(n_w, N_CORES):  # W'-cores: all masked
            slot_token[c] = msk[im : im + TOKC]
            im += TOKC
        for j, c in enumerate(range(n_w)):
            share = d // n_w + (1 if j < d % n_w else 0)
            sl = msk[im : im + share]
            im += share
            rest = unm[iu : iu + TOKC - share]
            iu += TOKC - share
            slot_token[c] = np.concatenate([sl, rest])
            mask_val[c, :share] = SCALING
    assert iu == len(unm) and im == len(msk)
    return slot_token, mask_val, core_w


def _prepare_in_maps(x, alora_offsets, W, b, A, B_w):
    bf = ml_dtypes.bfloat16
    f8 = ml_dtypes.float8_e4m3
    xf = np.asarray(x, dtype=np.float32).reshape(B * S, D_IN)
    W = np.asarray(W, dtype=np.float32)
    b = np.asarray(b, dtype=np.float32)
    A = np.asarray(A, dtype=np.float32)
    B_w = np.asarray(B_w, dtype=np.float32)
    offs = np.asarray(alora_offsets, dtype=np.int64)

    Wp = W + SCALING * (B_w @ A)
    slot_token, mask_val, core_w = _plan_permutation(offs)

    KSPLIT = KBF * P  # k-range split between bf16 and fp8

    def blockmajor(a, nblk):
        # [P, K, nblk*512] -> [P, nblk, K, 512] contiguous
        p, k, f = a.shape
        return np.ascontiguousarray(
            a.reshape(p, k, nblk, 512).transpose(0, 2, 1, 3)
        )

    def prep_w(Wm):
        Wt32 = Wm.T * WSCALE  # [D_IN, D_OUT]
        return (
            blockmajor(_tile_kx(Wt32[:KSPLIT], bf), NB),
            blockmajor(_tile_kx(Wt32[KSPLIT:], f8), NB),
        )

    wt_W, wt8_W = prep_w(W)
    wt_Wp, wt8_Wp = prep_w(Wp)
    at_np = _tile_kx(A.T, bf)  # [P, KT, R]
    bwt_np = (B_w.T * WSCALE).astype(bf)  # [R, D_OUT]
    bias_np = np.ascontiguousarray(
        np.broadcast_to((b * WSCALE).astype(bf), (P, D_OUT))
    )

    in_maps = []
    for c in range(N_CORES):
        xc = xf[slot_token[c]]  # [TOKC, D_IN]
        xt_np = blockmajor(_tile_kx(np.ascontiguousarray(xc[:, :KSPLIT].T), bf), SG)
        xt8_np = blockmajor(
            _tile_kx(np.ascontiguousarray(xc[:, KSPLIT:].T), f8), SG
        )
        xtf_np = _tile_kx(np.ascontiguousarray(xc[:FLEX].T), bf)
        maskv_np = np.ascontiguousarray(
            np.broadcast_to(mask_val[c].astype(bf), (P, FLEX))
        )
        wt_np, wt8_np = (wt_W, wt8_W) if core_w[c] else (wt_Wp, wt8_Wp)
        in_maps.append(
            {
                "xt": xt_np,
                "xt8": xt8_np,
                "xtf": xtf_np,
                "wt": wt_np,
                "wt8": wt8_np,
                "at": at_np,
                "bwt": bwt_np,
                "maskv": maskv_np,
                "bias": bias_np,
            }
        )
    return in_maps, slot_token


def _run(inputs: dict, trace: bool = False):
    from concourse.bass_utils import run_bass_kernel_spmd

    nc = _get_compiled()
    in_maps, slot_token = _prepare_in_maps(**inputs)
    res = None
    for attempt in range(3):
        try:
            res = run_bass_kernel_spmd(
                nc, in_maps, core_ids=list(range(N_CORES)), trace=trace
            )
            break
        except Exception:
            # transient device faults (e.g. NRT_EXEC_UNIT_UNRECOVERABLE)
            # clear on retry; re-raise only if persistent
            if attempt == 2:
                raise
    out = np.empty((B * S, D_OUT), dtype=np.float32)
    for c in range(N_CORES):
        out[slot_token[c]] = res.results[c]["out"]
    out /= WSCALE  # exact power-of-2 rescale of the scale-32 PSUM
    return out.reshape(B, S, D_OUT), res


def kernel(x, alora_offsets, W, b, A, B_w) -> np.ndarray:
    out, _ = _run(
        {"x": x, "alora_offsets": alora_offsets, "W": W, "b": b, "A": A, "B_w": B_w}
    )
    return out


# revision 19
# speedup vs baseline: 1.0374x; 1.0126x over previous
"""ALoraLinear on 8 TRN2 NeuronCores.

y = x @ W^T + b + mask ⊙ ((x @ A^T) @ B_w^T) * 2.0
  B=4, S=4096, D_IN=D_OUT=4096, R=32; mask = per-sample tail of the sequence.

Strategy (v3):
 1. Host folds the LoRA update into the weights: W' = W + 2·B_w@A. A token's
    output is x@W^T (unmasked) or x@W'^T (masked) — two dense GEMMs with no
    runtime LoRA path. Tokens are re-sorted host-side so each core is pure-W
    or pure-W' except ≤256 "minority" tokens per core, parked in 2 flex
    m-tiles and fixed up by a rank-32 correction with mask ∈ {0, ±2}.
 2. 10 of 32 k-subtiles run in fp8 e4m3 with perf_mode=DoubleRow: measured
    216 ns/MM at N=512 while contracting K=256 — a clean 2× over bf16.
    Emulated end-to-end rel err 0.0179 (gate 2e-2; HW matched emulation to
    <1e-4 at KF8=8).
 3. PSUM accumulates 32·y (weights pre-scaled ×32 so e4m3 sees RMS ~0.64
    instead of subnormal 0.02); bias (×32) is added by the vector engine at
    PSUM eviction; host divides the f32 output by 32 (exact).
 4. Quad super-groups: 4 m-tiles share one DR burst (DR-first, then 4×22
    bf16 k-MMs interleaved by k) — bf16↔fp8 mode transitions cost ~400 ns,
    amortized 4×. DR-first also gives the PE early work in the DMA-bound
    ramp; the x stream is issued in token-quarter passes matching the
    super-group consumption order.
"""

import numpy as np
import ml_dtypes

N_CORES = 8
B, S, D_IN, D_OUT, R = 4, 4096, 4096, 4096, 32
SCALING = 2.0
WSCALE = 32.0
P = 128
TOKC = (B * S) // N_CORES  # 2048 tokens per core
KT = D_IN // P  # 32 k-subtiles total
KF8 = 10  # k-subtiles in fp8 DoubleRow (must be even)
KBF = KT - KF8  # bf16 k-subtiles
NDR = KF8 // 2  # DoubleRow MMs per tile
NB = D_OUT // 512  # 8 n-blocks of 512
MT = TOKC // P  # 16 m-tiles of 128 tokens
SG = 4  # m-tiles per super-group (shared DR burst)
FLEXM = 2  # flex m-tiles (slots 0..255) carrying the LoRA fixup
FLEX = FLEXM * P
NW_CH = 3 if KBF % 3 == 0 else 2  # wt chunks per n-block
WCH = KBF // NW_CH  # bf16 k-subtiles per chunk DMA

_COMPILED = None


def _build():
    import concourse.bacc as bacc
    import concourse.mybir as mybir
    import concourse.tile as tile

    bf16 = mybir.dt.bfloat16
    f8 = mybir.dt.float8e4
    f32 = mybir.dt.float32
    DR = mybir.MatmulPerfMode.DoubleRow

    nc = bacc.Bacc("TRN2", target_bir_lowering=False, debug=False)

    # block-major layouts: every DMA below is per-partition contiguous
    xt_d = nc.dram_tensor("xt", [P, SG, KBF, 512], bf16, kind="ExternalInput")
    xt8_d = nc.dram_tensor("xt8", [P, SG, KF8, 512], f8, kind="ExternalInput")
    xtf_d = nc.dram_tensor("xtf", [P, KT, FLEX], bf16, kind="ExternalInput")
    wt_d = nc.dram_tensor("wt", [P, NB, KBF, 512], bf16, kind="ExternalInput")
    wt8_d = nc.dram_tensor("wt8", [P, NB, KF8, 512], f8, kind="ExternalInput")
    at_d = nc.dram_tensor("at", [P, KT, R], bf16, kind="ExternalInput")
    bwt_d = nc.dram_tensor("bwt", [R, D_OUT], bf16, kind="ExternalInput")
    maskv_d = nc.dram_tensor("maskv", [P, FLEX], bf16, kind="ExternalInput")
    bias_d = nc.dram_tensor("bias", [P, D_OUT], bf16, kind="ExternalInput")
    out_d = nc.dram_tensor("out", [TOKC, D_OUT], f32, kind="ExternalOutput")

    with tile.TileContext(nc) as tc:
        with (
            tc.tile_pool(name="const", bufs=1) as const,
            tc.tile_pool(name="xtp", bufs=1) as xtp,
            tc.tile_pool(name="wtp", bufs=NW_CH + 2) as wtp,
            tc.tile_pool(name="wt8p", bufs=2) as wt8p,
            tc.tile_pool(name="outp", bufs=4) as outp,
            tc.tile_pool(name="psum", bufs=7, space="PSUM") as psum,
            tc.tile_pool(name="psuma", bufs=1, space="PSUM") as psuma,
        ):
            at_sb = const.tile([P, KT, R], bf16, name="at_sb")
            xtf_sb = const.tile([P, KT, FLEX], bf16, name="xtf_sb")
            bwt_sb = const.tile([P, D_OUT], bf16, name="bwt_sb")
            maskv_sb = const.tile([P, FLEX], bf16, name="maskv_sb")
            bias_sb = const.tile([P, D_OUT], bf16, name="bias_sb")
            ut_sb = const.tile([P, FLEX], bf16, name="ut_sb")
            xt_sb = xtp.tile([P, SG, KBF, 512], bf16, name="xt_sb")
            xt8_sb = xtp.tile([P, SG, KF8, 512], f8, name="xt8_sb")

            def load_wt_chunk(n, c):
                wt = wtp.tile([P, WCH, 512], bf16, name="wt_sb")
                nc.sync.dma_start(
                    wt[:], wt_d.ap()[:, n : n + 1, c * WCH : (c + 1) * WCH, :]
                )
                return wt

            def load_wt8(n):
                w8 = wt8p.tile([P, KF8, 512], f8, name="wt8_sb")
                nc.sync.dma_start(w8[:], wt8_d.ap()[:, n : n + 1, :, :])
                return w8

            # PE clock warmup: the HAM gate holds the PE at half clock until
            # ~3.4us of sustained activity; the first ~6us are DMA-only.
            warm_sb = const.tile([P, FLEX], bf16, name="warm_sb")
            nc.gpsimd.memset(warm_sb[:], 0.0)
            wps = psuma.tile([R, FLEX], f32, name="aps")
            for i in range(32):
                nc.tensor.matmul(
                    wps[:],
                    warm_sb[:, 0:R],
                    warm_sb[:],
                    start=(i == 0),
                    stop=(i == 31),
                )

            # zero partition strips 32..127 of ut/bwt so the tail matmul sees
            # no SBUF garbage (NaN·0 = NaN); compute engines can't address
            # partition ranges starting mid-strip, so 3 strips of 32
            for p0 in (32, 64, 96):
                nc.vector.memset(ut_sb[p0 : p0 + 32, :], 0.0)
                nc.vector.memset(bwt_sb[p0 : p0 + 32, :], 0.0)

            # sync preamble in PE-need order: first DR operands for (q0,n0),
            # then act operands, then eviction-time operands (~30us in)
            nc.sync.dma_start(at_sb[:], at_d.ap()[:])
            wt8_0 = load_wt8(0)
            wt_chunks0 = [load_wt_chunk(0, 0)]
            nc.sync.dma_start(xtf_sb[:], xtf_d.ap()[:])
            nc.sync.dma_start(maskv_sb[:], maskv_d.ap()[:])
            wt_chunks0 += [load_wt_chunk(0, c) for c in range(1, NW_CH)]
            nc.sync.dma_start(bwt_sb[0:R, :], bwt_d.ap()[:])
            nc.sync.dma_start(bias_sb[:], bias_d.ap()[:])

            # gpsimd x stream: ONLY token quarter 0, k-granular so the ramp
            # chases it MM-by-MM. Quarters 1..3 are emitted onto the SYNC
            # queue interleaved between phase-1 weight loads (below), so
            # they don't compete with weights for HBM during the ramp —
            # they are not consumed until phase 2 (~190us in).
            nc.gpsimd.dma_start(xt8_sb[:, 0:1], xt8_d.ap()[:, 0:1])
            for k in range(KBF):
                nc.gpsimd.dma_start(
                    xt_sb[:, 0:1, k : k + 1, :], xt_d.ap()[:, 0:1, k : k + 1, :]
                )

            # LoRA activation for flex tokens only: u^T = A_pad @ x_flex^T,
            # one PSUM bank, then mask·u on the vector engine
            aps = psuma.tile([R, FLEX], f32, name="aps")
            for k in range(KT):
                nc.tensor.matmul(
                    aps[:],
                    at_sb[:, k, :],
                    xtf_sb[:, k, :],
                    start=(k == 0),
                    stop=(k == KT - 1),
                )
            nc.vector.tensor_mul(ut_sb[0:R, :], aps[:], maskv_sb[0:R, :])

            def super_group(q, n, chunks, w8):
                """4 m-tiles (q*SG .. q*SG+3): DR burst first, then bf16
                k-loops interleaved by k, then tails/evictions."""
                nsl = slice(n * 512, (n + 1) * 512)
                ms = [q * SG + i for i in range(SG)]
                ps = [psum.tile([P, 512], f32, name="ps") for _ in range(SG)]
                for i, m in enumerate(ms):
                    rsl = slice(i * P, (i + 1) * P)
                    for j in range(NDR):
                        nc.tensor.matmul(
                            ps[i][:],
                            xt8_sb[:, q, 2 * j : 2 * j + 2, rsl],
                            w8[:, 2 * j : 2 * j + 2, :],
                            start=(j == 0),
                            stop=False,
                            perf_mode=DR,
                        )
                for k in range(KBF):
                    for i, m in enumerate(ms):
                        nc.tensor.matmul(
                            ps[i][:],
                            xt_sb[:, q, k, i * P : (i + 1) * P],
                            chunks[k // WCH][:, k % WCH, :],
                            start=False,
                            stop=(k == KBF - 1 and m >= FLEXM),
                        )
                for i, m in enumerate(ms):
                    msl = slice(m * P, (m + 1) * P)
                    if m < FLEXM:
                        # rank-32 LoRA fixup for minority tokens (mask ∈ {0,±2})
                        nc.tensor.matmul(
                            ps[i][:], ut_sb[:, msl], bwt_sb[:, nsl],
                            start=False, stop=True,
                        )
                    ot = outp.tile([P, 512], f32, name="ot")
                    # eviction fuses the (×32-scaled) bias add
                    nc.vector.tensor_add(ot[:], ps[i][:], bias_sb[:, nsl])
                    # scalar engine issues output DMAs so their sem-waits
                    # never stall the sync engine's wt-prefetch stream
                    nc.scalar.dma_start(out_d.ap()[msl, nsl], ot[:])

            # phase 1: token quarter 0 across all n-blocks — needs only
            # 3.5MB of x up front, weights stream as consumed, so the PE
            # starts ~40us before the full x stream has landed
            super_group(0, 0, wt_chunks0, wt8_0)
            for n in range(1, NB):
                wt8_n = load_wt8(n)
                wt_chunks = [load_wt_chunk(n, c) for c in range(NW_CH)]
                if n == 1:
                    nc.sync.dma_start(xt8_sb[:, 1:SG], xt8_d.ap()[:, 1:SG])
                elif n in (2, 4, 6):
                    q = n // 2
                    nc.sync.dma_start(
                        xt_sb[:, q : q + 1], xt_d.ap()[:, q : q + 1]
                    )
                super_group(0, n, wt_chunks, wt8_n)
            # phase 2: quarters 1..3, n-outer (weights re-streamed once more;
            # ~27MB extra DMA, fully hidden under ~560us of PE work)
            for n in range(NB):
                wt8_n = load_wt8(n)
                wt_chunks = [load_wt_chunk(n, c) for c in range(NW_CH)]
                for q in range(1, MT // SG):
                    super_group(q, n, wt_chunks, wt8_n)

    nc.compile()
    return nc


def _get_compiled():
    global _COMPILED
    if _COMPILED is None:
        _COMPILED = _build()
    return _COMPILED


def _tile_kx(a_t: np.ndarray, dt) -> np.ndarray:
    """[K, F] -> partition-tiled [128, K/128, F], C-contiguous."""
    k, f = a_t.shape
    return np.ascontiguousarray(a_t.reshape(k // P, P, f).transpose(1, 0, 2)).astype(dt)


def _plan_permutation(offs):
    """Sort tokens so each core is pure-W or pure-W' except <=256 minority
    tokens parked in its first FLEX slots with mask ∈ {+2, -2}."""
    kk = np.minimum(offs, S)
    bnd = S - kk  # per-sample boundary; s >= bnd[i] is masked
    masked = np.zeros(B * S, dtype=bool)
    for i in range(B):
        masked[i * S + int(bnd[i]) : (i + 1) * S] = True
    unm = np.nonzero(~masked)[0]
    msk = np.nonzero(masked)[0]
    U = len(unm)

    n_w = None
    for cand in sorted(set([U // TOKC, -(-U // TOKC), round(U / TOKC)])):
        if cand < 0 or cand > N_CORES:
            continue
        delta = U - TOKC * cand
        if 0 <= delta <= FLEX * (N_CORES - cand) or (
            delta < 0 and -delta <= FLEX * cand
        ):
            n_w = cand
            break
    assert n_w is not None, f"no feasible core split for U={U}"
    delta = U - TOKC * n_w

    slot_token = np.empty((N_CORES, TOKC), dtype=np.int64)
    mask_val = np.zeros((N_CORES, FLEX), dtype=np.float32)
    core_w = np.zeros(N_CORES, dtype=bool)
    core_w[:n_w] = True

    iu = im = 0
    if delta >= 0:
        for c in range(n_w):  # W-cores: all unmasked
            slot_token[c] = unm[iu : iu + TOKC]
            iu += TOKC
        n_wp = N_CORES - n_w
        for j, c in enumerate(range(n_w, N_CORES)):
            share = delta // n_wp + (1 if j < delta % n_wp else 0)
            sl = unm[iu : iu + share]
            iu += share
            rest = msk[im : im + TOKC - share]
            im += TOKC - share
            slot_token[c] = np.concatenate([sl, rest])
            mask_val[c, :share] = -SCALING
    else:
        d = -delta
        for c in range(n_w, N_CORES):  # W'-cores: all masked
            slot_token[c] = msk[im : im + TOKC]
            im += TOKC
        for j, c in enumerate(range(n_w)):
            share = d // n_w + (1 if j < d % n_w else 0)
            sl = msk[im : im + share]
            im += share
            rest = unm[iu : iu + TOKC - share]
            iu += TOKC - share
            slot_token[c] = np.concatenate([sl, rest])
            mask_val[c, :share] = SCALING
    assert iu == len(unm) and im == len(msk)
    return slot_token, mask_val, core_w


def _prepare_in_maps(x, alora_offsets, W, b, A, B_w):
    bf = ml_dtypes.bfloat16
    f8 = ml_dtypes.float8_e4m3
    xf = np.asarray(x, dtype=np.float32).reshape(B * S, D_IN)
    W = np.asarray(W, dtype=np.float32)
    b = np.asarray(b, dtype=np.float32)
    A = np.asarray(A, dtype=np.float32)
    B_w = np.asarray(B_w, dtype=np.float32)
    offs = np.asarray(alora_offsets, dtype=np.int64)

    Wp = W + SCALING * (B_w @ A)
    slot_token, mask_val, core_w = _plan_permutation(offs)

    KSPLIT = KBF * P  # k-range split between bf16 and fp8

    def blockmajor(a, nblk):
        # [P, K, nblk*512] -> [P, nblk, K, 512] contiguous
        p, k, f = a.shape
        return np.ascontiguousarray(
            a.reshape(p, k, nblk, 512).transpose(0, 2, 1, 3)
        )

    def prep_w(Wm):
        Wt32 = Wm.T * WSCALE  # [D_IN, D_OUT]
        return (
            blockmajor(_tile_kx(Wt32[:KSPLIT], bf), NB),
            blockmajor(_tile_kx(Wt32[KSPLIT:], f8), NB),
        )

    wt_W, wt8_W = prep_w(W)
    wt_Wp, wt8_Wp = prep_w(Wp)
    at_np = _tile_kx(A.T, bf)  # [P, KT, R]
    bwt_np = (B_w.T * WSCALE).astype(bf)  # [R, D_OUT]
    bias_np = np.ascontiguousarray(
        np.broadcast_to((b * WSCALE).astype(bf), (P, D_OUT))
    )

    in_maps = []
    for c in range(N_CORES):
        xc = xf[slot_token[c]]  # [TOKC, D_IN]
        xt_np = blockmajor(_tile_kx(np.ascontiguousarray(xc[:, :KSPLIT].T), bf), SG)
        xt8_np = blockmajor(
            _tile_kx(np.ascontiguousarray(xc[:, KSPLIT:].T), f8), SG
        )
        xtf_np = _tile_kx(np.ascontiguousarray(xc[:FLEX].T), bf)
        maskv_np = np.ascontiguousarray(
            np.broadcast_to(mask_val[c].astype(bf), (P, FLEX))
        )
        wt_np, wt8_np = (wt_W, wt8_W) if core_w[c] else (wt_Wp, wt8_Wp)
        in_maps.append(
            {
                "xt": xt_np,
                "xt8": xt8_np,
                "xtf": xtf_np,
                "wt": wt_np,
                "wt8": wt8_np,
                "at": at_np,
                "bwt": bwt_np,
                "maskv": maskv_np,
                "bias": bias_np,
            }
        )
    return in_maps, slot_token


def _run(inputs: dict, trace: bool = False):
    from concourse.bass_utils import run_bass_kernel_spmd

    nc = _get_compiled()
    in_maps, slot_token = _prepare_in_maps(**inputs)
    res = None
    for attempt in range(3):
        try:
            res = run_bass_kernel_spmd(
                nc, in_maps, core_ids=list(range(N_CORES)), trace=trace
            )
            break
        except Exception:
            # transient device faults (e.g. NRT_EXEC_UNIT_UNRECOVERABLE)
            # clear on retry; re-raise only if persistent
            if attempt == 2:
                raise
    out = np.empty((B * S, D_OUT), dtype=np.float32)
    for c in range(N_CORES):
        out[slot_token[c]] = res.results[c]["out"]
    out /= WSCALE  # exact power-of-2 rescale of the scale-32 PSUM
    return out.reshape(B, S, D_OUT), res


def kernel(x, alora_offsets, W, b, A, B_w) -> np.ndarray:
    out, _ = _run(
        {"x": x, "alora_offsets": alora_offsets, "W": W, "b": b, "A": A, "B_w": B_w}
    )
    return out


# revision 20
# speedup vs baseline: 1.0805x; 1.0415x over previous
"""ALoraLinear on 8 TRN2 NeuronCores.

y = x @ W^T + b + mask ⊙ ((x @ A^T) @ B_w^T) * 2.0
  B=4, S=4096, D_IN=D_OUT=4096, R=32; mask = per-sample tail of the sequence.

Strategy (v3):
 1. Host folds the LoRA update into the weights: W' = W + 2·B_w@A. A token's
    output is x@W^T (unmasked) or x@W'^T (masked) — two dense GEMMs with no
    runtime LoRA path. Tokens are re-sorted host-side so each core is pure-W
    or pure-W' except ≤256 "minority" tokens per core, parked in 2 flex
    m-tiles and fixed up by a rank-32 correction with mask ∈ {0, ±2}.
 2. 10 of 32 k-subtiles run in fp8 e4m3 with perf_mode=DoubleRow: measured
    216 ns/MM at N=512 while contracting K=256 — a clean 2× over bf16.
    Emulated end-to-end rel err 0.0179 (gate 2e-2; HW matched emulation to
    <1e-4 at KF8=8).
 3. PSUM accumulates 32·y (weights pre-scaled ×32 so e4m3 sees RMS ~0.64
    instead of subnormal 0.02); bias (×32) is added by the vector engine at
    PSUM eviction; host divides the f32 output by 32 (exact).
 4. Quad super-groups: 4 m-tiles share one DR burst (DR-first, then 4×22
    bf16 k-MMs interleaved by k) — bf16↔fp8 mode transitions cost ~400 ns,
    amortized 4×. DR-first also gives the PE early work in the DMA-bound
    ramp; the x stream is issued in token-quarter passes matching the
    super-group consumption order.
"""

import numpy as np
import ml_dtypes

N_CORES = 8
B, S, D_IN, D_OUT, R = 4, 4096, 4096, 4096, 32
SCALING = 2.0
WSCALE = 32.0
P = 128
TOKC = (B * S) // N_CORES  # 2048 tokens per core
KT = D_IN // P  # 32 k-subtiles total
KF8 = 12  # k-subtiles in fp8 DoubleRow (must be even)
KBF = KT - KF8  # bf16 k-subtiles
NDR = KF8 // 2  # DoubleRow MMs per tile
NB = D_OUT // 512  # 8 n-blocks of 512
MT = TOKC // P  # 16 m-tiles of 128 tokens
SG = 4  # m-tiles per super-group (shared DR burst)
FLEXM = 2  # flex m-tiles (slots 0..255) carrying the LoRA fixup
FLEX = FLEXM * P
NW_CH = 3 if KBF % 3 == 0 else 2  # wt chunks per n-block
WCH = KBF // NW_CH  # bf16 k-subtiles per chunk DMA

_COMPILED = None


def _build():
    import concourse.bacc as bacc
    import concourse.mybir as mybir
    import concourse.tile as tile

    bf16 = mybir.dt.bfloat16
    f8 = mybir.dt.float8e4
    f32 = mybir.dt.float32
    DR = mybir.MatmulPerfMode.DoubleRow

    nc = bacc.Bacc("TRN2", target_bir_lowering=False, debug=False)

    # block-major layouts: every DMA below is per-partition contiguous
    xt_d = nc.dram_tensor("xt", [P, SG, KBF, 512], bf16, kind="ExternalInput")
    xt8_d = nc.dram_tensor("xt8", [P, SG, KF8, 512], f8, kind="ExternalInput")
    xtf_d = nc.dram_tensor("xtf", [P, KT, FLEX], bf16, kind="ExternalInput")
    wt_d = nc.dram_tensor("wt", [P, NB, KBF, 512], bf16, kind="ExternalInput")
    wt8_d = nc.dram_tensor("wt8", [P, NB, KF8, 512], f8, kind="ExternalInput")
    at_d = nc.dram_tensor("at", [P, KT, R], bf16, kind="ExternalInput")
    bwt_d = nc.dram_tensor("bwt", [R, D_OUT], bf16, kind="ExternalInput")
    maskv_d = nc.dram_tensor("maskv", [P, FLEX], bf16, kind="ExternalInput")
    bias_d = nc.dram_tensor("bias", [P, D_OUT], bf16, kind="ExternalInput")
    out_d = nc.dram_tensor("out", [TOKC, D_OUT], f32, kind="ExternalOutput")

    with tile.TileContext(nc) as tc:
        with (
            tc.tile_pool(name="const", bufs=1) as const,
            tc.tile_pool(name="xtp", bufs=1) as xtp,
            tc.tile_pool(name="wtp", bufs=NW_CH + 2) as wtp,
            tc.tile_pool(name="wt8p", bufs=2) as wt8p,
            tc.tile_pool(name="outp", bufs=4) as outp,
            tc.tile_pool(name="psum", bufs=7, space="PSUM") as psum,
            tc.tile_pool(name="psuma", bufs=1, space="PSUM") as psuma,
        ):
            at_sb = const.tile([P, KT, R], bf16, name="at_sb")
            xtf_sb = const.tile([P, KT, FLEX], bf16, name="xtf_sb")
            bwt_sb = const.tile([P, D_OUT], bf16, name="bwt_sb")
            maskv_sb = const.tile([P, FLEX], bf16, name="maskv_sb")
            bias_sb = const.tile([P, D_OUT], bf16, name="bias_sb")
            ut_sb = const.tile([P, FLEX], bf16, name="ut_sb")
            xt_sb = xtp.tile([P, SG, KBF, 512], bf16, name="xt_sb")
            xt8_sb = xtp.tile([P, SG, KF8, 512], f8, name="xt8_sb")

            def load_wt_chunk(n, c):
                wt = wtp.tile([P, WCH, 512], bf16, name="wt_sb")
                nc.sync.dma_start(
                    wt[:], wt_d.ap()[:, n : n + 1, c * WCH : (c + 1) * WCH, :]
                )
                return wt

            def load_wt8(n):
                w8 = wt8p.tile([P, KF8, 512], f8, name="wt8_sb")
                nc.sync.dma_start(w8[:], wt8_d.ap()[:, n : n + 1, :, :])
                return w8

            # PE clock warmup: the HAM gate holds the PE at half clock until
            # ~3.4us of sustained activity; the first ~6us are DMA-only.
            warm_sb = const.tile([P, FLEX], bf16, name="warm_sb")
            nc.gpsimd.memset(warm_sb[:], 0.0)
            wps = psuma.tile([R, FLEX], f32, name="aps")
            for i in range(32):
                nc.tensor.matmul(
                    wps[:],
                    warm_sb[:, 0:R],
                    warm_sb[:],
                    start=(i == 0),
                    stop=(i == 31),
                )

            # zero partition strips 32..127 of ut/bwt so the tail matmul sees
            # no SBUF garbage (NaN·0 = NaN); compute engines can't address
            # partition ranges starting mid-strip, so 3 strips of 32
            for p0 in (32, 64, 96):
                nc.vector.memset(ut_sb[p0 : p0 + 32, :], 0.0)
                nc.vector.memset(bwt_sb[p0 : p0 + 32, :], 0.0)

            # sync preamble in PE-need order: first DR operands for (q0,n0),
            # then act operands, then eviction-time operands (~30us in)
            nc.sync.dma_start(at_sb[:], at_d.ap()[:])
            wt8_0 = load_wt8(0)
            wt_chunks0 = [load_wt_chunk(0, 0)]
            nc.sync.dma_start(xtf_sb[:], xtf_d.ap()[:])
            nc.sync.dma_start(maskv_sb[:], maskv_d.ap()[:])
            wt_chunks0 += [load_wt_chunk(0, c) for c in range(1, NW_CH)]
            nc.sync.dma_start(bwt_sb[0:R, :], bwt_d.ap()[:])
            nc.sync.dma_start(bias_sb[:], bias_d.ap()[:])

            # gpsimd x stream: ONLY token quarter 0, k-granular so the ramp
            # chases it MM-by-MM. Quarters 1..3 are emitted onto the SYNC
            # queue interleaved between phase-1 weight loads (below), so
            # they don't compete with weights for HBM during the ramp —
            # they are not consumed until phase 2 (~190us in).
            nc.gpsimd.dma_start(xt8_sb[:, 0:1], xt8_d.ap()[:, 0:1])
            for k in range(KBF):
                nc.gpsimd.dma_start(
                    xt_sb[:, 0:1, k : k + 1, :], xt_d.ap()[:, 0:1, k : k + 1, :]
                )

            # LoRA activation for flex tokens only: u^T = A_pad @ x_flex^T,
            # one PSUM bank, then mask·u on the vector engine
            aps = psuma.tile([R, FLEX], f32, name="aps")
            for k in range(KT):
                nc.tensor.matmul(
                    aps[:],
                    at_sb[:, k, :],
                    xtf_sb[:, k, :],
                    start=(k == 0),
                    stop=(k == KT - 1),
                )
            nc.vector.tensor_mul(ut_sb[0:R, :], aps[:], maskv_sb[0:R, :])

            def super_group(q, n, chunks, w8):
                """4 m-tiles (q*SG .. q*SG+3): DR burst first, then bf16
                k-loops interleaved by k, then tails/evictions."""
                nsl = slice(n * 512, (n + 1) * 512)
                ms = [q * SG + i for i in range(SG)]
                ps = [psum.tile([P, 512], f32, name="ps") for _ in range(SG)]
                for i, m in enumerate(ms):
                    rsl = slice(i * P, (i + 1) * P)
                    for j in range(NDR):
                        nc.tensor.matmul(
                            ps[i][:],
                            xt8_sb[:, q, 2 * j : 2 * j + 2, rsl],
                            w8[:, 2 * j : 2 * j + 2, :],
                            start=(j == 0),
                            stop=False,
                            perf_mode=DR,
                        )
                for k in range(KBF):
                    for i, m in enumerate(ms):
                        nc.tensor.matmul(
                            ps[i][:],
                            xt_sb[:, q, k, i * P : (i + 1) * P],
                            chunks[k // WCH][:, k % WCH, :],
                            start=False,
                            stop=(k == KBF - 1 and m >= FLEXM),
                        )
                for i, m in enumerate(ms):
                    msl = slice(m * P, (m + 1) * P)
                    if m < FLEXM:
                        # rank-32 LoRA fixup for minority tokens (mask ∈ {0,±2})
                        nc.tensor.matmul(
                            ps[i][:], ut_sb[:, msl], bwt_sb[:, nsl],
                            start=False, stop=True,
                        )
                    ot = outp.tile([P, 512], f32, name="ot")
                    # eviction fuses the (×32-scaled) bias add
                    nc.vector.tensor_add(ot[:], ps[i][:], bias_sb[:, nsl])
                    # scalar engine issues output DMAs so their sem-waits
                    # never stall the sync engine's wt-prefetch stream
                    nc.scalar.dma_start(out_d.ap()[msl, nsl], ot[:])

            # phase 1: token quarter 0 across all n-blocks — needs only
            # 3.5MB of x up front, weights stream as consumed, so the PE
            # starts ~40us before the full x stream has landed
            super_group(0, 0, wt_chunks0, wt8_0)
            for n in range(1, NB):
                wt8_n = load_wt8(n)
                wt_chunks = [load_wt_chunk(n, c) for c in range(NW_CH)]
                if n == 1:
                    nc.sync.dma_start(xt8_sb[:, 1:SG], xt8_d.ap()[:, 1:SG])
                elif n in (2, 4, 6):
                    q = n // 2
                    nc.sync.dma_start(
                        xt_sb[:, q : q + 1], xt_d.ap()[:, q : q + 1]
                    )
                super_group(0, n, wt_chunks, wt8_n)
            # phase 2: quarters 1..3, n-outer (weights re-streamed once more;
            # ~27MB extra DMA, fully hidden under ~560us of PE work)
            for n in range(NB):
                wt8_n = load_wt8(n)
                wt_chunks = [load_wt_chunk(n, c) for c in range(NW_CH)]
                for q in range(1, MT // SG):
                    super_group(q, n, wt_chunks, wt8_n)

    nc.compile()
    return nc


def _get_compiled():
    global _COMPILED
    if _COMPILED is None:
        _COMPILED = _build()
    return _COMPILED


def _tile_kx(a_t: np.ndarray, dt) -> np.ndarray:
    """[K, F] -> partition-tiled [128, K/128, F], C-contiguous."""
    k, f = a_t.shape
    return np.ascontiguousarray(a_t.reshape(k // P, P, f).transpose(1, 0, 2)).astype(dt)


def _plan_permutation(offs):
    """Sort tokens so each core is pure-W or pure-W' except <=256 minority
    tokens parked in its first FLEX slots with mask ∈ {+2, -2}."""
    kk = np.minimum(offs, S)
    bnd = S - kk  # per-sample boundary; s >= bnd[i] is masked
    masked = np.zeros(B * S, dtype=bool)
    for i in range(B):
        masked[i * S + int(bnd[i]) : (i + 1) * S] = True
    unm = np.nonzero(~masked)[0]
    msk = np.nonzero(masked)[0]
    U = len(unm)

    n_w = None
    for cand in sorted(set([U // TOKC, -(-U // TOKC), round(U / TOKC)])):
        if cand < 0 or cand > N_CORES:
            continue
        delta = U - TOKC * cand
        if 0 <= delta <= FLEX * (N_CORES - cand) or (
            delta < 0 and -delta <= FLEX * cand
        ):
            n_w = cand
            break
    assert n_w is not None, f"no feasible core split for U={U}"
    delta = U - TOKC * n_w

    slot_token = np.empty((N_CORES, TOKC), dtype=np.int64)
    mask_val = np.zeros((N_CORES, FLEX), dtype=np.float32)
    core_w = np.zeros(N_CORES, dtype=bool)
    core_w[:n_w] = True

    iu = im = 0
    if delta >= 0:
        for c in range(n_w):  # W-cores: all unmasked
            slot_token[c] = unm[iu : iu + TOKC]
            iu += TOKC
        n_wp = N_CORES - n_w
        for j, c in enumerate(range(n_w, N_CORES)):
            share = delta // n_wp + (1 if j < delta % n_wp else 0)
            sl = unm[iu : iu + share]
            iu += share
            rest = msk[im : im + TOKC - share]
            im += TOKC - share
            slot_token[c] = np.concatenate([sl, rest])
            mask_val[c, :share] = -SCALING
    else:
        d = -delta
        for c in range(n_w, N_CORES):  # W'-cores: all masked
            slot_token[c] = msk[im : im + TOKC]
            im += TOKC
        for j, c in enumerate(range(n_w)):
            share = d // n_w + (1 if j < d % n_w else 0)
            sl = msk[im : im + share]
            im += share
            rest = unm[iu : iu + TOKC - share]
            iu += TOKC - share
            slot_token[c] = np.concatenate([sl, rest])
            mask_val[c, :share] = SCALING
    assert iu == len(unm) and im == len(msk)
    return slot_token, mask_val, core_w


def _prepare_in_maps(x, alora_offsets, W, b, A, B_w):
    bf = ml_dtypes.bfloat16
    f8 = ml_dtypes.float8_e4m3
    xf = np.asarray(x, dtype=np.float32).reshape(B * S, D_IN)
    W = np.asarray(W, dtype=np.float32)
    b = np.asarray(b, dtype=np.float32)
    A = np.asarray(A, dtype=np.float32)
    B_w = np.asarray(B_w, dtype=np.float32)
    offs = np.asarray(alora_offsets, dtype=np.int64)

    Wp = W + SCALING * (B_w @ A)
    slot_token, mask_val, core_w = _plan_permutation(offs)

    KSPLIT = KBF * P  # k-range split between bf16 and fp8

    def blockmajor(a, nblk):
        # [P, K, nblk*512] -> [P, nblk, K, 512] contiguous
        p, k, f = a.shape
        return np.ascontiguousarray(
            a.reshape(p, k, nblk, 512).transpose(0, 2, 1, 3)
        )

    def prep_w(Wm):
        Wt32 = Wm.T * WSCALE  # [D_IN, D_OUT]
        return (
            blockmajor(_tile_kx(Wt32[:KSPLIT], bf), NB),
            blockmajor(_tile_kx(Wt32[KSPLIT:], f8), NB),
        )

    wt_W, wt8_W = prep_w(W)
    wt_Wp, wt8_Wp = prep_w(Wp)
    at_np = _tile_kx(A.T, bf)  # [P, KT, R]
    bwt_np = (B_w.T * WSCALE).astype(bf)  # [R, D_OUT]
    bias_np = np.ascontiguousarray(
        np.broadcast_to((b * WSCALE).astype(bf), (P, D_OUT))
    )

    in_maps = []
    for c in range(N_CORES):
        xc = xf[slot_token[c]]  # [TOKC, D_IN]
        xt_np = blockmajor(_tile_kx(np.ascontiguousarray(xc[:, :KSPLIT].T), bf), SG)
        xt8_np = blockmajor(
            _tile_kx(np.ascontiguousarray(xc[:, KSPLIT:].T), f8), SG
        )
        xtf_np = _tile_kx(np.ascontiguousarray(xc[:FLEX].T), bf)
        maskv_np = np.ascontiguousarray(
            np.broadcast_to(mask_val[c].astype(bf), (P, FLEX))
        )
        wt_np, wt8_np = (wt_W, wt8_W) if core_w[c] else (wt_Wp, wt8_Wp)
        in_maps.append(
            {
                "xt": xt_np,
                "xt8": xt8_np,
                "xtf": xtf_np,
                "wt": wt_np,
                "wt8": wt8_np,
                "at": at_np,
                "bwt": bwt_np,
                "maskv": maskv_np,
                "bias": bias_np,
            }
        )
    return in_maps, slot_token


def _run(inputs: dict, trace: bool = False):
    from concourse.bass_utils import run_bass_kernel_spmd

    nc = _get_compiled()
    in_maps, slot_token = _prepare_in_maps(**inputs)
    res = None
    for attempt in range(3):
        try:
            res = run_bass_kernel_spmd(
                nc, in_maps, core_ids=list(range(N_CORES)), trace=trace
            )
            break
        except Exception:
            # transient device faults (e.g. NRT_EXEC_UNIT_UNRECOVERABLE)
            # clear on retry; re-raise only if persistent
            if attempt == 2:
                raise
    out = np.empty((B * S, D_OUT), dtype=np.float32)
    for c in range(N_CORES):
        out[slot_token[c]] = res.results[c]["out"]
    out /= WSCALE  # exact power-of-2 rescale of the scale-32 PSUM
    return out.reshape(B, S, D_OUT), res


def kernel(x, alora_offsets, W, b, A, B_w) -> np.ndarray:
    out, _ = _run(
        {"x": x, "alora_offsets": alora_offsets, "W": W, "b": b, "A": A, "B_w": B_w}
    )
    return out


# revision 26
# speedup vs baseline: 1.0848x; 1.0040x over previous
"""ALoraLinear on 8 TRN2 NeuronCores.

y = x @ W^T + b + mask ⊙ ((x @ A^T) @ B_w^T) * 2.0
  B=4, S=4096, D_IN=D_OUT=4096, R=32; mask = per-sample tail of the sequence.

Strategy (v3):
 1. Host folds the LoRA update into the weights: W' = W + 2·B_w@A. A token's
    output is x@W^T (unmasked) or x@W'^T (masked) — two dense GEMMs with no
    runtime LoRA path. Tokens are re-sorted host-side so each core is pure-W
    or pure-W' except ≤256 "minority" tokens per core, parked in 2 flex
    m-tiles and fixed up by a rank-32 correction with mask ∈ {0, ±2}.
 2. 10 of 32 k-subtiles run in fp8 e4m3 with perf_mode=DoubleRow: measured
    216 ns/MM at N=512 while contracting K=256 — a clean 2× over bf16.
    Emulated end-to-end rel err 0.0179 (gate 2e-2; HW matched emulation to
    <1e-4 at KF8=8).
 3. PSUM accumulates 32·y (weights pre-scaled ×32 so e4m3 sees RMS ~0.64
    instead of subnormal 0.02); bias (×32) is added by the vector engine at
    PSUM eviction; host divides the f32 output by 32 (exact).
 4. Quad super-groups: 4 m-tiles share one DR burst (DR-first, then 4×22
    bf16 k-MMs interleaved by k) — bf16↔fp8 mode transitions cost ~400 ns,
    amortized 4×. DR-first also gives the PE early work in the DMA-bound
    ramp; the x stream is issued in token-quarter passes matching the
    super-group consumption order.
"""

import numpy as np
import ml_dtypes

N_CORES = 8
B, S, D_IN, D_OUT, R = 4, 4096, 4096, 4096, 32
SCALING = 2.0
WSCALE = 32.0
P = 128
TOKC = (B * S) // N_CORES  # 2048 tokens per core
KT = D_IN // P  # 32 k-subtiles total
KF8 = 12  # k-subtiles in fp8 DoubleRow (must be even)
KBF = KT - KF8  # bf16 k-subtiles
NDR = KF8 // 2  # DoubleRow MMs per tile
NB = D_OUT // 512  # 8 n-blocks of 512
MT = TOKC // P  # 16 m-tiles of 128 tokens
SG = 4  # m-tiles per super-group (shared DR burst)
FLEXM = 2  # flex m-tiles (slots 0..255) carrying the LoRA fixup
FLEX = FLEXM * P
NW_CH = 3 if KBF % 3 == 0 else 2  # wt chunks per n-block
WCH = KBF // NW_CH  # bf16 k-subtiles per chunk DMA

_COMPILED = None


def _build():
    import concourse.bacc as bacc
    import concourse.mybir as mybir
    import concourse.tile as tile

    bf16 = mybir.dt.bfloat16
    f8 = mybir.dt.float8e4
    f32 = mybir.dt.float32
    DR = mybir.MatmulPerfMode.DoubleRow

    nc = bacc.Bacc("TRN2", target_bir_lowering=False, debug=False)

    # block-major layouts: every DMA below is per-partition contiguous
    xt_d = nc.dram_tensor("xt", [P, SG, KBF, 512], bf16, kind="ExternalInput")
    xt8_d = nc.dram_tensor("xt8", [P, SG, KF8, 512], f8, kind="ExternalInput")
    xtf_d = nc.dram_tensor("xtf", [P, KT, FLEX], bf16, kind="ExternalInput")
    wt_d = nc.dram_tensor("wt", [P, NB, KBF, 512], bf16, kind="ExternalInput")
    wt8_d = nc.dram_tensor("wt8", [P, NB, KF8, 512], f8, kind="ExternalInput")
    at_d = nc.dram_tensor("at", [P, KT, R], bf16, kind="ExternalInput")
    bwt_d = nc.dram_tensor("bwt", [R, D_OUT], bf16, kind="ExternalInput")
    maskv_d = nc.dram_tensor("maskv", [P, FLEX], bf16, kind="ExternalInput")
    bias_d = nc.dram_tensor("bias", [P, D_OUT], bf16, kind="ExternalInput")
    out_d = nc.dram_tensor("out", [TOKC, D_OUT], f32, kind="ExternalOutput")

    with tile.TileContext(nc) as tc:
        with (
            tc.tile_pool(name="const", bufs=1) as const,
            tc.tile_pool(name="xtp", bufs=1) as xtp,
            tc.tile_pool(name="wtp", bufs=NW_CH + 2) as wtp,
            tc.tile_pool(name="wt8p", bufs=2) as wt8p,
            tc.tile_pool(name="outp", bufs=4) as outp,
            tc.tile_pool(name="psum", bufs=7, space="PSUM") as psum,
            tc.tile_pool(name="psuma", bufs=1, space="PSUM") as psuma,
        ):
            at_sb = const.tile([P, KT, R], bf16, name="at_sb")
            xtf_sb = const.tile([P, KT, FLEX], bf16, name="xtf_sb")
            bwt_sb = const.tile([P, D_OUT], bf16, name="bwt_sb")
            maskv_sb = const.tile([P, FLEX], bf16, name="maskv_sb")
            bias_sb = const.tile([P, D_OUT], bf16, name="bias_sb")
            ut_sb = const.tile([P, FLEX], bf16, name="ut_sb")
            xt_sb = xtp.tile([P, SG, KBF, 512], bf16, name="xt_sb")
            xt8_sb = xtp.tile([P, SG, KF8, 512], f8, name="xt8_sb")

            def load_wt_chunk(n, c):
                wt = wtp.tile([P, WCH, 512], bf16, name="wt_sb")
                nc.sync.dma_start(
                    wt[:], wt_d.ap()[:, n : n + 1, c * WCH : (c + 1) * WCH, :]
                )
                return wt

            def load_wt8(n):
                w8 = wt8p.tile([P, KF8, 512], f8, name="wt8_sb")
                nc.sync.dma_start(w8[:], wt8_d.ap()[:, n : n + 1, :, :])
                return w8

            # PE clock warmup: the HAM gate holds the PE at half clock until
            # ~3.4us of sustained activity; the first ~6us are DMA-only.
            # PE clock warmup reads at_sb — the first DMA to land (~0.7us) —
            # so the PE starts almost immediately; results are discarded
            # (the bank is reset by act's start=True)
            wps = psuma.tile([R, FLEX], f32, name="aps")

            # zero partition strips 32..127 of ut/bwt so the tail matmul sees
            # no SBUF garbage (NaN·0 = NaN); compute engines can't address
            # partition ranges starting mid-strip, so 3 strips of 32
            for p0 in (32, 64, 96):
                nc.vector.memset(ut_sb[p0 : p0 + 32, :], 0.0)
                nc.vector.memset(bwt_sb[p0 : p0 + 32, :], 0.0)

            # sync preamble in PE-need order: first DR operands for (q0,n0),
            # then act operands, then eviction-time operands (~30us in)
            nc.sync.dma_start(at_sb[:], at_d.ap()[:])
            for i in range(32):
                nc.tensor.matmul(
                    wps[:],
                    at_sb[:, 0, :],
                    at_sb[:, 0:8, :],
                    start=(i == 0),
                    stop=(i == 31),
                )
            wt8_0 = load_wt8(0)
            wt_chunks0 = [load_wt_chunk(0, 0)]
            nc.sync.dma_start(xtf_sb[:], xtf_d.ap()[:])
            nc.sync.dma_start(maskv_sb[:], maskv_d.ap()[:])
            wt_chunks0 += [load_wt_chunk(0, c) for c in range(1, NW_CH)]
            nc.sync.dma_start(bwt_sb[0:R, :], bwt_d.ap()[:])
            nc.sync.dma_start(bias_sb[:], bias_d.ap()[:])

            # gpsimd x stream: ONLY token quarter 0, k-granular so the ramp
            # chases it MM-by-MM. Quarters 1..3 are emitted onto the SYNC
            # queue interleaved between phase-1 weight loads (below), so
            # they don't compete with weights for HBM during the ramp —
            # they are not consumed until phase 2 (~190us in).
            nc.gpsimd.dma_start(xt8_sb[:, 0:1], xt8_d.ap()[:, 0:1])
            for k in range(KBF):
                nc.gpsimd.dma_start(
                    xt_sb[:, 0:1, k : k + 1, :], xt_d.ap()[:, 0:1, k : k + 1, :]
                )

            # LoRA activation for flex tokens only: u^T = A_pad @ x_flex^T,
            # one PSUM bank, then mask·u on the vector engine
            aps = psuma.tile([R, FLEX], f32, name="aps")
            for k in range(KT):
                nc.tensor.matmul(
                    aps[:],
                    at_sb[:, k, :],
                    xtf_sb[:, k, :],
                    start=(k == 0),
                    stop=(k == KT - 1),
                )
            nc.vector.tensor_mul(ut_sb[0:R, :], aps[:], maskv_sb[0:R, :])

            def super_group(q, n, chunks, w8, stagger=False):
                """4 m-tiles (q*SG .. q*SG+3): DR burst first, then bf16
                k-loops interleaved by k, then tails/evictions. With
                stagger=True (last super only) the m-tiles run sequentially
                so evictions overlap compute instead of serializing at the
                very end of the kernel."""
                nsl = slice(n * 512, (n + 1) * 512)
                ms = [q * SG + i for i in range(SG)]
                ps = [psum.tile([P, 512], f32, name="ps") for _ in range(SG)]

                def dr_burst(i):
                    rsl = slice(i * P, (i + 1) * P)
                    for j in range(NDR):
                        nc.tensor.matmul(
                            ps[i][:],
                            xt8_sb[:, q, 2 * j : 2 * j + 2, rsl],
                            w8[:, 2 * j : 2 * j + 2, :],
                            start=(j == 0),
                            stop=False,
                            perf_mode=DR,
                        )

                def bf16_mm(i, m, k):
                    nc.tensor.matmul(
                        ps[i][:],
                        xt_sb[:, q, k, i * P : (i + 1) * P],
                        chunks[k // WCH][:, k % WCH, :],
                        start=False,
                        stop=(k == KBF - 1 and m >= FLEXM),
                    )

                def tail_evict(i, m):
                    msl = slice(m * P, (m + 1) * P)
                    if m < FLEXM:
                        # rank-32 LoRA fixup for minority tokens (mask ∈ {0,±2})
                        nc.tensor.matmul(
                            ps[i][:], ut_sb[:, msl], bwt_sb[:, nsl],
                            start=False, stop=True,
                        )
                    ot = outp.tile([P, 512], f32, name="ot")
                    # eviction fuses the (×32-scaled) bias add
                    nc.vector.tensor_add(ot[:], ps[i][:], bias_sb[:, nsl])
                    # scalar engine issues output DMAs so their sem-waits
                    # never stall the sync engine's wt-prefetch stream
                    nc.scalar.dma_start(out_d.ap()[msl, nsl], ot[:])

                if stagger:
                    for i, m in enumerate(ms):
                        dr_burst(i)
                        for k in range(KBF):
                            bf16_mm(i, m, k)
                        tail_evict(i, m)
                    return
                for i, m in enumerate(ms):
                    dr_burst(i)
                for k in range(KBF):
                    for i, m in enumerate(ms):
                        bf16_mm(i, m, k)
                for i, m in enumerate(ms):
                    tail_evict(i, m)

            # phase 1: token quarter 0 across all n-blocks — needs only
            # 3.5MB of x up front, weights stream as consumed, so the PE
            # starts ~40us before the full x stream has landed
            super_group(0, 0, wt_chunks0, wt8_0)
            for n in range(1, NB):
                wt8_n = load_wt8(n)
                wt_chunks = [load_wt_chunk(n, c) for c in range(NW_CH)]
                if n == 1:
                    nc.sync.dma_start(xt8_sb[:, 1:SG], xt8_d.ap()[:, 1:SG])
                elif n in (2, 4, 6):
                    q = n // 2
                    nc.sync.dma_start(
                        xt_sb[:, q : q + 1], xt_d.ap()[:, q : q + 1]
                    )
                super_group(0, n, wt_chunks, wt8_n)
            # phase 2: quarters 1..3, n-outer (weights re-streamed once more;
            # ~27MB extra DMA, fully hidden under ~560us of PE work)
            for n in range(NB):
                wt8_n = load_wt8(n)
                wt_chunks = [load_wt_chunk(n, c) for c in range(NW_CH)]
                for q in range(1, MT // SG):
                    last = n == NB - 1 and q == MT // SG - 1
                    super_group(q, n, wt_chunks, wt8_n, stagger=last)

    nc.compile()
    return nc


def _get_compiled():
    global _COMPILED
    if _COMPILED is None:
        _COMPILED = _build()
    return _COMPILED


def _tile_kx(a_t: np.ndarray, dt) -> np.ndarray:
    """[K, F] -> partition-tiled [128, K/128, F], C-contiguous."""
    k, f = a_t.shape
    return np.ascontiguousarray(a_t.reshape(k // P, P, f).transpose(1, 0, 2)).astype(dt)


def _plan_permutation(offs):
    """Sort tokens so each core is pure-W or pure-W' except <=256 minority
    tokens parked in its first FLEX slots with mask ∈ {+2, -2}."""
    kk = np.minimum(offs, S)
    bnd = S - kk  # per-sample boundary; s >= bnd[i] is masked
    masked = np.zeros(B * S, dtype=bool)
    for i in range(B):
        masked[i * S + int(bnd[i]) : (i + 1) * S] = True
    unm = np.nonzero(~masked)[0]
    msk = np.nonzero(masked)[0]
    U = len(unm)

    n_w = None
    for cand in sorted(set([U // TOKC, -(-U // TOKC), round(U / TOKC)])):
        if cand < 0 or cand > N_CORES:
            continue
        delta = U - TOKC * cand
        if 0 <= delta <= FLEX * (N_CORES - cand) or (
            delta < 0 and -delta <= FLEX * cand
        ):
            n_w = cand
            break
    assert n_w is not None, f"no feasible core split for U={U}"
    delta = U - TOKC * n_w

    slot_token = np.empty((N_CORES, TOKC), dtype=np.int64)
    mask_val = np.zeros((N_CORES, FLEX), dtype=np.float32)
    core_w = np.zeros(N_CORES, dtype=bool)
    core_w[:n_w] = True

    iu = im = 0
    if delta >= 0:
        for c in range(n_w):  # W-cores: all unmasked
            slot_token[c] = unm[iu : iu + TOKC]
            iu += TOKC
        n_wp = N_CORES - n_w
        for j, c in enumerate(range(n_w, N_CORES)):
            share = delta // n_wp + (1 if j < delta % n_wp else 0)
            sl = unm[iu : iu + share]
            iu += share
            rest = msk[im : im + TOKC - share]
            im += TOKC - share
            slot_token[c] = np.concatenate([sl, rest])
            mask_val[c, :share] = -SCALING
    else:
        d = -delta
        for c in range(n_w, N_CORES):  # W'-cores: all masked
            slot_token[c] = msk[im : im + TOKC]
            im += TOKC
        for j, c in enumerate(range(n_w)):
            share = d // n_w + (1 if j < d % n_w else 0)
            sl = msk[im : im + share]
            im += share
            rest = unm[iu : iu + TOKC - share]
            iu += TOKC - share
            slot_token[c] = np.concatenate([sl, rest])
            mask_val[c, :share] = SCALING
    assert iu == len(unm) and im == len(msk)
    return slot_token, mask_val, core_w


def _prepare_in_maps(x, alora_offsets, W, b, A, B_w):
    bf = ml_dtypes.bfloat16
    f8 = ml_dtypes.float8_e4m3
    xf = np.asarray(x, dtype=np.float32).reshape(B * S, D_IN)
    W = np.asarray(W, dtype=np.float32)
    b = np.asarray(b, dtype=np.float32)
    A = np.asarray(A, dtype=np.float32)
    B_w = np.asarray(B_w, dtype=np.float32)
    offs = np.asarray(alora_offsets, dtype=np.int64)

    Wp = W + SCALING * (B_w @ A)
    slot_token, mask_val, core_w = _plan_permutation(offs)

    KSPLIT = KBF * P  # k-range split between bf16 and fp8

    def blockmajor(a, nblk):
        # [P, K, nblk*512] -> [P, nblk, K, 512] contiguous
        p, k, f = a.shape
        return np.ascontiguousarray(
            a.reshape(p, k, nblk, 512).transpose(0, 2, 1, 3)
        )

    def prep_w(Wm):
        Wt32 = Wm.T * WSCALE  # [D_IN, D_OUT]
        return (
            blockmajor(_tile_kx(Wt32[:KSPLIT], bf), NB),
            blockmajor(_tile_kx(Wt32[KSPLIT:], f8), NB),
        )

    wt_W, wt8_W = prep_w(W)
    wt_Wp, wt8_Wp = prep_w(Wp)
    at_np = _tile_kx(A.T, bf)  # [P, KT, R]
    bwt_np = (B_w.T * WSCALE).astype(bf)  # [R, D_OUT]
    bias_np = np.ascontiguousarray(
        np.broadcast_to((b * WSCALE).astype(bf), (P, D_OUT))
    )

    in_maps = []
    for c in range(N_CORES):
        xc = xf[slot_token[c]]  # [TOKC, D_IN]
        xt_np = blockmajor(_tile_kx(np.ascontiguousarray(xc[:, :KSPLIT].T), bf), SG)
        xt8_np = blockmajor(
            _tile_kx(np.ascontiguousarray(xc[:, KSPLIT:].T), f8), SG
        )
        xtf_np = _tile_kx(np.ascontiguousarray(xc[:FLEX].T), bf)
        maskv_np = np.ascontiguousarray(
            np.broadcast_to(mask_val[c].astype(bf), (P, FLEX))
        )
        wt_np, wt8_np = (wt_W, wt8_W) if core_w[c] else (wt_Wp, wt8_Wp)
        in_maps.append(
            {
                "xt": xt_np,
                "xt8": xt8_np,
                "xtf": xtf_np,
                "wt": wt_np,
                "wt8": wt8_np,
                "at": at_np,
                "bwt": bwt_np,
                "maskv": maskv_np,
                "bias": bias_np,
            }
        )
    return in_maps, slot_token


def _run(inputs: dict, trace: bool = False):
    from concourse.bass_utils import run_bass_kernel_spmd

    nc = _get_compiled()
    in_maps, slot_token = _prepare_in_maps(**inputs)
    res = None
    for attempt in range(3):
        try:
            res = run_bass_kernel_spmd(
                nc, in_maps, core_ids=list(range(N_CORES)), trace=trace
            )
            break
        except Exception:
            # transient device faults (e.g. NRT_EXEC_UNIT_UNRECOVERABLE)
            # clear on retry; re-raise only if persistent
            if attempt == 2:
                raise
    out = np.empty((B * S, D_OUT), dtype=np.float32)
    for c in range(N_CORES):
        out[slot_token[c]] = res.results[c]["out"]
    out /= WSCALE  # exact power-of-2 rescale of the scale-32 PSUM
    return out.reshape(B, S, D_OUT), res


def kernel(x, alora_offsets, W, b, A, B_w) -> np.ndarray:
    out, _ = _run(
        {"x": x, "alora_offsets": alora_offsets, "W": W, "b": b, "A": A, "B_w": B_w}
    )
    return out
